# revision 16
# baseline (speedup 1.0000x reference)
"""Trainium2 Bass kernel for nn_Attention_19404662243470.

Sharding: 8 cores = (batch 2) x (heads 4). Each core computes the full
attention pipeline for its (b, h) pair in transposed layout [d, n]; the
final pointwise conv partials are ReduceScattered within each batch's
4-core group, and LayerNorm2d runs on each core's position shard.
"""

import numpy as np

import concourse.bass as bass
import concourse.tile as tile
from concourse import bacc, mybir
from concourse.bass_utils import run_bass_kernel_spmd

dt = mybir.dt
AF = mybir.ActivationFunctionType
OP = mybir.AluOpType

B, DIM, Hs, Ws = 2, 128, 64, 64
HEADS, DH = 4, 64
N = Hs * Ws  # 4096
EPS = 1e-6
IC = 512  # i-chunk width
NIC = N // IC  # 8
JB = 128  # j-block
NJB = N // JB  # 32
GRP = 3  # j-blocks per exp batch (3 PSUM banks)
NCH = N // 128  # 32 (128-position chunks)
G = Hs + 2  # 66 padded grid


def _build():
    nc = bacc.Bacc()

    def par(name, shape, dtyp=dt.float32):
        return nc.declare_dram_parameter(name, list(shape), dtyp, isOutput=False)

    x = par("x", [DIM, N])
    meshb = par("meshb", [3, N], dt.bfloat16)
    pewT = par("pewT", [3, DIM], dt.bfloat16)
    qdiags = par("qdiags", [DIM, 9 * DIM], dt.float32r)
    pwq = par("pwq", [DIM, DH], dt.float32r)
    pwk = par("pwk", [DIM, DH], dt.float32r)
    pwv = par("pwv", [DIM, DH], dt.float32r)
    pwvb = par("pwvb", [DIM, DH], dt.bfloat16)
    lnqw = par("lnqw", [1, DH], dt.float32r)
    lnqb = par("lnqb", [DH, 1])
    lnkw = par("lnkw", [1, DH], dt.float32r)
    lnkb = par("lnkb", [DH, 1])
    lnow = par("lnow", [1, DH], dt.float32r)
    lnob = par("lnob", [DH, 1])
    odiags = par("odiags", [DH, 9 * DH], dt.bfloat16)
    opw = par("opw", [DH, DIM], dt.bfloat16)
    ln2w = par("ln2w", [1, DIM])
    ln2b = par("ln2b", [1, DIM])
    o64ad = par("o64a", [DIM, 1], dt.float32r)
    o64bd = par("o64b", [DIM, 1], dt.float32r)
    zpad = par("zpad", [DIM, G], dt.float32r)
    ones1d = par("ones1", [1, DH], dt.float32r)
    out_ext = nc.declare_dram_parameter("out", [N // 4, DIM], dt.float32, isOutput=True)

    rs_in = nc.dram_tensor("rs_in", [N, DIM], dt.float32)
    rs_out = nc.dram_tensor("rs_out", [N // 4, DIM], dt.float32)

    with (
        nc.allow_low_precision(reason="float32r compute by design"),
        tile.TileContext(nc) as tc,
        tc.tile_pool(name="main", bufs=1) as main,
        tc.tile_pool(name="tmp2", bufs=2) as tmp2,
    ):
        # ---- persistent SBUF tiles ----
        ST2 = main.tile([128, N], dt.float32r)  # raw q (or k) rows 0-63, squares 64-127
        QL = main.tile([DH, N], dt.float32r)
        KL = main.tile([DH, N], dt.float32r)
        VT = main.tile([DH, N], dt.bfloat16)  # v^T for the skip connection
        V = main.tile([128, NCH, DH + 1], dt.bfloat16)  # v-tiles + ones row col
        OS = main.tile([128, N], dt.float32r)  # attn out rows 0-63, squares 64-127
        SC = main.tile([1, 3 * N], dt.float32r)  # slots: mu, E2, scratch
        REC = main.tile([1, N], dt.float32r)
        o64ab = main.tile([128, 2], dt.float32r)
        nc.sync.dma_start(out=o64ab[:, 0:1], in_=o64ad[:, :])
        nc.sync.dma_start(out=o64ab[:, 1:2], in_=o64bd[:, :])
        lnqb_t = main.tile([DH, 1], dt.float32)
        lnkb_t = main.tile([DH, 1], dt.float32)
        lnob_t = main.tile([DH, 1], dt.float32)
        nc.sync.dma_start(out=lnqb_t, in_=lnqb[:, :])
        nc.sync.dma_start(out=lnkb_t, in_=lnkb[:, :])
        nc.sync.dma_start(out=lnob_t, in_=lnob[:, :])
        lnqw_t = main.tile([1, DH], dt.float32r)
        lnkw_t = main.tile([1, DH], dt.float32r)
        lnow_t = main.tile([1, DH], dt.float32r)
        nc.sync.dma_start(out=lnqw_t, in_=lnqw[:, :])
        nc.sync.dma_start(out=lnkw_t, in_=lnkw[:, :])
        nc.sync.dma_start(out=lnow_t, in_=lnow[:, :])
        nc.vector.memset(V, 1.0)
        epsP = main.tile([128, 1], dt.float32)
        nc.vector.memset(epsP, EPS)

        # ============ Stage A: pos + depthwise + pointwise + q/k LN ============
        with tc.tile_pool(name="stageA", bufs=1) as pA:
            Xg = pA.tile([DIM, G, G], dt.float32r)
            nc.sync.dma_start(out=Xg[:, 0:1, :], in_=zpad[:, :].unsqueeze(1))
            nc.sync.dma_start(out=Xg[:, G - 1 : G, :], in_=zpad[:, :].unsqueeze(1))
            nc.sync.dma_start(out=Xg[:, 1 : G - 1, 0:1], in_=zpad[:, 0 : G - 2].unsqueeze(2))
            nc.sync.dma_start(out=Xg[:, 1 : G - 1, G - 1 : G], in_=zpad[:, 0 : G - 2].unsqueeze(2))
            for q in range(4):
                nc.sync.dma_start(
                    out=Xg[:, 1 + 16 * q : 1 + 16 * (q + 1), 1 : 1 + Ws],
                    in_=x[:, 1024 * q : 1024 * (q + 1)]
                    .bitcast(dt.float32r)
                    .rearrange("p (a b) -> p a b", b=Ws),
                )
            msh = pA.tile([3, N], dt.bfloat16)
            nc.sync.dma_start(out=msh, in_=meshb[:, :])
            pwt = pA.tile([3, DIM], dt.bfloat16)
            nc.sync.dma_start(out=pwt, in_=pewT[:, :])
            qdg = pA.tile([DIM, 9, DIM], dt.float32r)
            nc.sync.dma_start(
                out=qdg, in_=qdiags[:, :].rearrange("p (t c) -> p t c", t=9)
            )
            Yr = pA.tile([DIM, N], dt.float32r)
            Yb = pA.tile([DIM, N], dt.bfloat16)
            pwq_t = pA.tile([DIM, DH], dt.float32r)
            pwk_t = pA.tile([DIM, DH], dt.float32r)
            pwv_t = pA.tile([DIM, DH], dt.float32r)
            pwvb_t = pA.tile([DIM, DH], dt.bfloat16)
            nc.sync.dma_start(out=pwq_t, in_=pwq[:, :])
            nc.sync.dma_start(out=pwk_t, in_=pwk[:, :])
            nc.sync.dma_start(out=pwv_t, in_=pwv[:, :])
            nc.sync.dma_start(out=pwvb_t, in_=pwvb[:, :])

            # positional encoding into the guard interior: x += pe_w @ mesh + pe_b
            psA1 = tc.tile_pool(name="psA1", bufs=2, space="PSUM")
            psA = psA1.__enter__()
            for c in range(NIC):
                pos_ps = psA.tile([DIM, IC], dt.float32, tag="pos")
                nc.tensor.matmul(
                    pos_ps, pwt, msh[:, c * IC : (c + 1) * IC], start=True, stop=True
                )
                r0 = c * 8
                view = Xg[:, 1 + r0 : 9 + r0, 1 : 1 + Ws]
                nc.vector.tensor_add(
                    out=view,
                    in0=view,
                    in1=pos_ps.rearrange("p (a b) -> p a b", b=Ws),
                )

            # depthwise 3x3 as 9 accumulated diag matmuls; then pointwise
            for c in range(NIC):
                dwp = psA.tile([DIM, IC], dt.float32, tag="dw")
                r0 = c * 8
                t = 0
                for di in range(3):
                    for dj in range(3):
                        nc.tensor.matmul(
                            dwp,
                            qdg[:, t, :],
                            Xg[:, r0 + di : r0 + di + 8, dj : dj + Ws],
                            start=(t == 0),
                            stop=(t == 8),
                        )
                        t += 1
                nc.vector.tensor_copy(out=Yr[:, c * IC : (c + 1) * IC], in_=dwp)
                nc.vector.tensor_copy(out=Yb[:, c * IC : (c + 1) * IC], in_=dwp)

            psA1.__exit__(None, None, None)
            psA2 = tc.tile_pool(name="psA2", bufs=1, space="PSUM")
            psA = psA2.__enter__()
            # v in N-layout (for AV lhsT): v[n,d] tiles
            for ch in range(NCH):
                vp = psA.tile([128, DH], dt.float32, tag="vp", bufs=2)
                nc.tensor.matmul(
                    vp,
                    Yb[:, ch * 128 : (ch + 1) * 128],
                    pwvb_t,
                    start=True,
                    stop=True,
                )
                nc.vector.tensor_copy(out=V[:, ch, 0:DH], in_=vp)

            # q/k/v pointwise in T-layout + per-head LN for q and k
            def head_ln(pw_t, w_row, b_t, dest):
                for c in range(NIC):
                    qp = psA.tile([DH, IC], dt.float32, tag="qp", bufs=2)
                    nc.tensor.matmul(
                        qp, pw_t, Yr[:, c * IC : (c + 1) * IC], start=True, stop=True
                    )
                    nc.vector.tensor_copy(
                        out=ST2[0:DH, c * IC : (c + 1) * IC], in_=qp
                    )
                    nc.vector.tensor_mul(
                        out=ST2[DH:128, c * IC : (c + 1) * IC],
                        in0=ST2[0:DH, c * IC : (c + 1) * IC],
                        in1=ST2[0:DH, c * IC : (c + 1) * IC],
                    )
                # stats: mu and E[x^2] rows via ones-matmuls
                for c in range(NIC):
                    smu = psA.tile([1, IC], dt.float32, tag="smu", bufs=1)
                    se2 = psA.tile([1, IC], dt.float32, tag="se2", bufs=1)
                    nc.tensor.matmul(
                        smu, o64ab[:, 0:1], ST2[:, c * IC : (c + 1) * IC], start=True, stop=True
                    )
                    nc.tensor.matmul(
                        se2, o64ab[:, 1:2], ST2[:, c * IC : (c + 1) * IC], start=True, stop=True
                    )
                    nc.vector.tensor_copy(out=SC[:, c * IC : (c + 1) * IC], in_=smu)
                    nc.vector.tensor_copy(
                        out=SC[:, N + c * IC : N + (c + 1) * IC], in_=se2
                    )
                mu = SC[:, 0:N]
                e2 = SC[:, N : 2 * N]
                w_ = SC[:, 2 * N : 3 * N]
                # w = -mu*mu ; w = E2 + w = var; w = sqrt(var+eps); w = 1/w
                nc.vector.scalar_tensor_tensor(
                    out=w_, in0=mu, scalar=-1.0, in1=mu, op0=OP.mult, op1=OP.mult
                )
                nc.vector.tensor_add(out=w_, in0=e2, in1=w_)
                nc.scalar.activation(out=w_, in_=w_, func=AF.Sqrt, bias=epsP[0:1, :])
                nc.vector.reciprocal(out=w_, in_=w_)  # rs
                nc.vector.tensor_mul(out=mu, in0=mu, in1=w_)  # mu*rs (mu dead)
                # broadcast (w*rs) and (w*mu*rs) then apply
                for c in range(NIC):
                    bcA = psA.tile([DH, IC], dt.float32, tag="bcA")
                    bcB = psA.tile([DH, IC], dt.float32, tag="bcB")
                    nc.tensor.matmul(
                        bcA, w_row, w_[:, c * IC : (c + 1) * IC], start=True, stop=True
                    )
                    nc.tensor.matmul(
                        bcB, w_row, mu[:, c * IC : (c + 1) * IC], start=True, stop=True
                    )
                    T = tmp2.tile([DH, IC], dt.float32, tag="T")
                    nc.vector.tensor_mul(
                        out=T, in0=ST2[0:DH, c * IC : (c + 1) * IC], in1=bcA
                    )
                    nc.vector.scalar_tensor_tensor(
                        out=dest[:, c * IC : (c + 1) * IC],
                        in0=T,
                        scalar=b_t,
                        in1=bcB,
                        op0=OP.add,
                        op1=OP.subtract,
                    )

            head_ln(pwq_t, lnqw_t, lnqb_t, QL)
            head_ln(pwk_t, lnkw_t, lnkb_t, KL)

            # v^T in T-layout for the skip connection
            for c in range(NIC):
                qp = psA.tile([DH, IC], dt.float32, tag="qp", bufs=2)
                nc.tensor.matmul(
                    qp, pwv_t, Yr[:, c * IC : (c + 1) * IC], start=True, stop=True
                )
                nc.vector.tensor_copy(out=VT[:, c * IC : (c + 1) * IC], in_=qp)
            psA2.__exit__(None, None, None)

        # ============ Stage B: attention ============
        ones1 = main.tile([1, DH], dt.float32r)
        nc.sync.dma_start(out=ones1, in_=ones1d[:, :])
        groups = []
        jb = 0
        while jb < NJB:
            n = min(GRP, NJB - jb)
            groups.append((jb, n))
            jb += n
        avs_tiles = []
        with tc.tile_pool(name="psB", bufs=1, space="PSUM") as psB, tc.tile_pool(
            name="sbB", bufs=3
        ) as sbB:
            for c in range(NIC):
                avp = psB.tile([DH + 1, IC], dt.float32, tag="avp", bufs=1)
                stgs = {}
                Es = {}

                def issue_st(gi, c=c):
                    jb0, ng = groups[gi]
                    stg = psB.tile([128, GRP * IC], dt.float32, tag="stg", bufs=2)
                    for t in range(ng):
                        j0 = (jb0 + t) * JB
                        nc.tensor.matmul(
                            stg[:, t * IC : (t + 1) * IC],
                            KL[:, j0 : j0 + JB],
                            QL[:, c * IC : (c + 1) * IC],
                            start=True,
                            stop=True,
                        )
                    stgs[gi] = stg

                def issue_exp(gi):
                    jb0, ng = groups[gi]
                    E = sbB.tile([128, GRP * IC], dt.bfloat16, tag="E")
                    nc.scalar.activation(
                        out=E[:, 0 : ng * IC],
                        in_=stgs.pop(gi)[:, 0 : ng * IC],
                        func=AF.Exp,
                        scale=float(DH**-0.5),
                    )
                    Es[gi] = E

                def issue_av(gi, c=c):
                    jb0, ng = groups[gi]
                    E = Es.pop(gi)
                    for t in range(ng):
                        nc.tensor.matmul(
                            avp,
                            V[:, jb0 + t, :],
                            E[:, t * IC : (t + 1) * IC],
                            start=(jb0 + t == 0),
                            stop=(jb0 + t == NJB - 1),
                            skip_group_check=True,
                        )

                issue_st(0)
                issue_exp(0)
                for gi in range(1, len(groups)):
                    issue_st(gi)
                    issue_exp(gi)
                    issue_av(gi - 1)
                issue_av(len(groups) - 1)
                # collect softmax denominator row for this chunk
                nc.vector.tensor_copy(
                    out=REC[:, c * IC : (c + 1) * IC], in_=avp[DH : DH + 1, :]
                )
                AVS = sbB.tile([DH, IC], dt.float32, tag="AVS", bufs=8)
                nc.vector.tensor_copy(out=AVS, in_=avp[0:DH, :])
                avs_tiles.append(AVS)
            nc.vector.reciprocal(out=REC, in_=REC)
            for c in range(NIC):
                bcR = psB.tile([DH, IC], dt.float32, tag="bcR", bufs=1)
                nc.tensor.matmul(
                    bcR, ones1, REC[:, c * IC : (c + 1) * IC], start=True, stop=True
                )
                nc.vector.tensor_mul(
                    out=OS[0:DH, c * IC : (c + 1) * IC], in0=avs_tiles[c], in1=bcR
                )
                nc.vector.tensor_add(
                    out=OS[0:DH, c * IC : (c + 1) * IC],
                    in0=OS[0:DH, c * IC : (c + 1) * IC],
                    in1=VT[:, c * IC : (c + 1) * IC],
                )
                nc.vector.tensor_mul(
                    out=OS[DH:128, c * IC : (c + 1) * IC],
                    in0=OS[0:DH, c * IC : (c + 1) * IC],
                    in1=OS[0:DH, c * IC : (c + 1) * IC],
                )

        # ============ Stage C        # ============ Stage C: out-LN + depthwise + pointwise partial ============
        with (
            tc.tile_pool(name="stageC", bufs=1) as pC,
            tc.tile_pool(name="psC", bufs=1, space="PSUM") as psC,
        ):
            Og = pC.tile([DH, G, G], dt.bfloat16)
            nc.vector.memset(Og, 0.0)
            # out-LN stats
            for c in range(NIC):
                smu = psC.tile([1, IC], dt.float32, tag="smu", bufs=1)
                se2 = psC.tile([1, IC], dt.float32, tag="se2", bufs=1)
                nc.tensor.matmul(
                    smu, o64ab[:, 0:1], OS[:, c * IC : (c + 1) * IC], start=True, stop=True
                )
                nc.tensor.matmul(
                    se2, o64ab[:, 1:2], OS[:, c * IC : (c + 1) * IC], start=True, stop=True
                )
                nc.vector.tensor_copy(out=SC[:, c * IC : (c + 1) * IC], in_=smu)
                nc.vector.tensor_copy(
                    out=SC[:, N + c * IC : N + (c + 1) * IC], in_=se2
                )
            mu = SC[:, 0:N]
            e2 = SC[:, N : 2 * N]
            w_ = SC[:, 2 * N : 3 * N]
            nc.vector.scalar_tensor_tensor(
                out=w_, in0=mu, scalar=-1.0, in1=mu, op0=OP.mult, op1=OP.mult
            )
            nc.vector.tensor_add(out=w_, in0=e2, in1=w_)
            nc.scalar.activation(out=w_, in_=w_, func=AF.Sqrt, bias=epsP[0:1, :])
            nc.vector.reciprocal(out=w_, in_=w_)
            nc.vector.tensor_mul(out=mu, in0=mu, in1=w_)
            for c in range(NIC):
                bcA = psC.tile([DH, IC], dt.float32, tag="bcA")
                bcB = psC.tile([DH, IC], dt.float32, tag="bcB")
                nc.tensor.matmul(
                    bcA, lnow_t, w_[:, c * IC : (c + 1) * IC], start=True, stop=True
                )
                nc.tensor.matmul(
                    bcB, lnow_t, mu[:, c * IC : (c + 1) * IC], start=True, stop=True
                )
                T = tmp2.tile([DH, IC], dt.float32, tag="T")
                nc.vector.tensor_mul(
                    out=T, in0=OS[0:DH, c * IC : (c + 1) * IC], in1=bcA
                )
                r0 = c * 8
                nc.vector.scalar_tensor_tensor(
                    out=Og[:, 1 + r0 : 9 + r0, 1 : 1 + Ws],
                    in0=T.rearrange("p (a b) -> p a b", b=Ws),
                    scalar=lnob_t,
                    in1=bcB.rearrange("p (a b) -> p a b", b=Ws),
                    op0=OP.add,
                    op1=OP.subtract,
                )

            odg = pC.tile([DH, 9, DH], dt.bfloat16)
            nc.sync.dma_start(
                out=odg, in_=odiags[:, :].rearrange("p (t c) -> p t c", t=9)
            )
            opw_t = pC.tile([DH, DIM], dt.bfloat16)
            nc.sync.dma_start(out=opw_t, in_=opw[:, :])
            DWO = pC.tile([DH, N], dt.bfloat16)
            for c in range(NIC):
                dwp = psC.tile([DH, IC], dt.float32, tag="dw", bufs=2)
                r0 = c * 8
                t = 0
                for di in range(3):
                    for dj in range(3):
                        nc.tensor.matmul(
                            dwp,
                            odg[:, t, :],
                            Og[:, r0 + di : r0 + di + 8, dj : dj + Ws],
                            start=(t == 0),
                            stop=(t == 8),
                        )
                        t += 1
                nc.vector.tensor_copy(out=DWO[:, c * IC : (c + 1) * IC], in_=dwp)
            # partial^T [pos, chan] then DMA to rs_in
            for ch in range(NCH):
                pp = psC.tile([128, DIM], dt.float32, tag="pp", bufs=2)
                nc.tensor.matmul(
                    pp, DWO[:, ch * 128 : (ch + 1) * 128], opw_t, start=True, stop=True
                )
                PP = tmp2.tile([128, DIM], dt.float32, tag="PP")
                nc.vector.tensor_copy(out=PP, in_=pp)
                nc.sync.dma_start(out=rs_in[ch * 128 : (ch + 1) * 128, :], in_=PP)

        # ============ Stage D: ReduceScatter + LayerNorm2d ============
        nc.gpsimd.collective_compute(
            "ReduceScatter",
            OP.add,
            replica_groups=[[0, 1, 2, 3], [4, 5, 6, 7]],
            ins=[rs_in[:, :]],
            outs=[rs_out[:, :]],
        )
        with tc.tile_pool(name="stageD", bufs=2) as pD:
            w_b = pD.tile([128, DIM], dt.float32, bufs=1)
            b_b = pD.tile([128, DIM], dt.float32, bufs=1)
            nc.sync.dma_start(out=w_b, in_=ln2w[:, :].to_broadcast([128, DIM]))
            nc.sync.dma_start(out=b_b, in_=ln2b[:, :].to_broadcast([128, DIM]))
            for tkn in range(8):
                R = pD.tile([128, DIM], dt.float32, tag="R")
                nc.sync.dma_start(out=R, in_=rs_out[tkn * 128 : (tkn + 1) * 128, :])
                st = pD.tile([128, 6], dt.float32, tag="st")
                nc.vector.bn_stats(out=st, in_=R)
                mv = pD.tile([128, 2], dt.float32, tag="mv")
                nc.vector.bn_aggr(out=mv, in_=st)
                sd = pD.tile([128, 1], dt.float32, tag="sd")
                nc.scalar.activation(
                    out=sd, in_=mv[:, 1:2], func=AF.Sqrt, bias=epsP
                )
                nc.vector.reciprocal(out=sd, in_=sd)
                nc.vector.tensor_scalar(
                    out=R,
                    in0=R,
                    scalar1=mv[:, 0:1],
                    scalar2=sd,
                    op0=OP.subtract,
                    op1=OP.mult,
                )
                R2 = pD.tile([128, DIM], dt.float32, tag="R2")
                nc.vector.tensor_mul(out=R2, in0=R, in1=w_b)
                nc.vector.tensor_add(out=R2, in0=R2, in1=b_b)
                nc.sync.dma_start(
                    out=out_ext[tkn * 128 : (tkn + 1) * 128, :], in_=R2
                )

    return nc


_cached = {}


def _get_nc():
    if "nc" not in _cached:
        nc = _build()
        nc.finalize()
        _cached["nc"] = nc
    return _cached["nc"]


def _make_in_maps(inputs):
    import ml_dtypes

    x = np.asarray(inputs["x"], np.float32)
    pe_w = np.asarray(inputs["pe_w"], np.float32)
    pe_b = np.asarray(inputs["pe_b"], np.float32)
    qkv_dw = np.asarray(inputs["qkv_dw"], np.float32)
    qkv_pw = np.asarray(inputs["qkv_pw"], np.float32)
    out_dw = np.asarray(inputs["out_dw"], np.float32)
    out_pw = np.asarray(inputs["out_pw"], np.float32)
    nq_w, nq_b = np.asarray(inputs["nq_w"], np.float32), np.asarray(
        inputs["nq_b"], np.float32
    )
    nk_w, nk_b = np.asarray(inputs["nk_w"], np.float32), np.asarray(
        inputs["nk_b"], np.float32
    )
    no_w, no_b = np.asarray(inputs["no_w"], np.float32), np.asarray(
        inputs["no_b"], np.float32
    )
    ln_w, ln_b = np.asarray(inputs["ln_w"], np.float32), np.asarray(
        inputs["ln_b"], np.float32
    )

    gx = np.linspace(0.0, 1.0, Hs, dtype=np.float32)
    gy = np.linspace(0.0, 1.0, Ws, dtype=np.float32)
    meshb = np.stack(
        [
            np.repeat(gx, Ws),
            np.tile(gy, Hs),
            np.ones(N, np.float32),
        ]
    ).astype(ml_dtypes.bfloat16)
    pewT = np.stack([pe_w[:, 0], pe_w[:, 1], pe_b]).astype(ml_dtypes.bfloat16)

    idx = np.arange(DH)
    in_maps = []
    for c in range(8):
        b, h = c // 4, c % 4
        rows = h + HEADS * idx
        qdiags = np.zeros((DIM, 9, DIM), np.float32)
        taps = qkv_dw.reshape(DIM, 9)
        for t in range(9):
            qdiags[np.arange(DIM), t, np.arange(DIM)] = taps[:, t]
        odiags = np.zeros((DH, 9, DH), np.float32)
        otaps = out_dw[rows].reshape(DH, 9)
        for t in range(9):
            odiags[idx, t, idx] = otaps[:, t]
        m = {
            "x": np.ascontiguousarray(x[b].reshape(DIM, N)),
            "meshb": meshb,
            "pewT": pewT,
            "qdiags": np.ascontiguousarray(qdiags.reshape(DIM, 9 * DIM)),
            "pwq": np.ascontiguousarray(qkv_pw[rows, :].T),
            "pwk": np.ascontiguousarray(qkv_pw[DIM * 2 + rows, :].T),
            "pwv": np.ascontiguousarray(qkv_pw[DIM * 4 + rows, :].T),
            "pwvb": np.ascontiguousarray(qkv_pw[DIM * 4 + rows, :].T).astype(
                ml_dtypes.bfloat16
            ),
            "lnqw": np.ascontiguousarray(nq_w[h][None, :]),
            "lnqb": np.ascontiguousarray(nq_b[h][:, None]),
            "lnkw": np.ascontiguousarray(nk_w[h][None, :]),
            "lnkb": np.ascontiguousarray(nk_b[h][:, None]),
            "lnow": np.ascontiguousarray(no_w[h][None, :]),
            "lnob": np.ascontiguousarray(no_b[h][:, None]),
            "odiags": np.ascontiguousarray(odiags.reshape(DH, 9 * DH)).astype(
                ml_dtypes.bfloat16
            ),
            "opw": np.ascontiguousarray(out_pw[:, rows].T).astype(ml_dtypes.bfloat16),
            "ln2w": np.ascontiguousarray(ln_w[None, :]),
            "ln2b": np.ascontiguousarray(ln_b[None, :]),
            "o64a": np.concatenate([np.full(64, 1.0 / DH, np.float32), np.zeros(64, np.float32)])[:, None],
            "o64b": np.concatenate([np.zeros(64, np.float32), np.full(64, 1.0 / DH, np.float32)])[:, None],
            "zpad": np.zeros((DIM, G), np.float32),
            "ones1": np.ones((1, DH), np.float32),
        }
        in_maps.append(m)
    return in_maps


def run_on_device(inputs, **kw):
    nc = _get_nc()
    in_maps = _make_in_maps(inputs)
    res = run_bass_kernel_spmd(nc, in_maps, core_ids=list(range(8)), **kw)
    out = np.zeros((B, DIM, N), np.float32)
    for c in range(8):
        b, h = c // 4, c % 4
        out[b][:, h * (N // 4) : (h + 1) * (N // 4)] = res.results[c]["out"].T
    return out.reshape(B, DIM, Hs, Ws), res


def kernel(**inputs):
    out, _ = run_on_device(inputs)
    return out


# revision 23
# speedup vs baseline: 1.2731x; 1.2731x over previous
"""Trainium2 Bass kernel for nn_Attention_19404662243470.

Sharding: 8 cores = (batch 2) x (heads 4). Each core computes the full
attention pipeline for its (b, h) pair in transposed layout [d, n]; the
final pointwise conv partials are ReduceScattered within each batch's
4-core group, and LayerNorm2d runs on each core's position shard.

Layout notes:
 - q/k/v come out of the pointwise conv directly as [d, n] ("T layout"),
   which is exactly the operand layout the S^T = K Q^T matmul needs.
 - softmax runs without max-subtraction (logits are bounded ~|5|); the
   denominator falls out of the AV matmul via an appended ones-row in V.
 - per-head LN over d (the partition dim) uses ones-matmuls for the
   stats and K=1 broadcast matmuls to spread per-column scalars.
 - S^T pairs are row-packed onto the two halves of the PE array
   (contraction is only 64 deep), doubling S^T throughput.
 - long PE idle gaps are avoided (HAM throttles the PE clock to 1.2 GHz
   after ~3.4us of idle and has been seen never to recover): LN scalar
   chains are overlapped with independent matmul work, and the out-LN
   is folded into the per-chunk attention loop.
"""

import numpy as np

import concourse.bass as bass
import concourse.tile as tile
from concourse import bacc, mybir
from concourse.bass_utils import run_bass_kernel_spmd

dt = mybir.dt
AF = mybir.ActivationFunctionType
OP = mybir.AluOpType

B, DIM, Hs, Ws = 2, 128, 64, 64
HEADS, DH = 4, 64
N = Hs * Ws  # 4096
EPS = 1e-6
IC = 512  # i-chunk width
NIC = N // IC  # 8
JB = 128  # j-block
NJB = N // JB  # 32
NCH = N // 128  # 32
G = Hs + 2  # 66 padded grid


def _build():
    nc = bacc.Bacc()

    def par(name, shape, dtyp=dt.float32):
        return nc.declare_dram_parameter(name, list(shape), dtyp, isOutput=False)

    x = par("x", [DIM, N])
    meshb = par("meshb", [3, N], dt.bfloat16)
    pewT = par("pewT", [3, DIM], dt.bfloat16)
    qdiags = par("qdiags", [DIM, 9 * DIM], dt.float32r)
    pwq = par("pwq", [DIM, DH], dt.float32r)
    pwk = par("pwk", [DIM, DH], dt.float32r)
    pwv = par("pwv", [DIM, DH], dt.float32r)
    lnqw = par("lnqw", [1, DH], dt.float32r)
    lnqb = par("lnqb", [DH, 1])
    lnkw = par("lnkw", [1, DH], dt.float32r)
    lnkb = par("lnkb", [DH, 1])
    lnow = par("lnow", [1, DH], dt.float32r)
    lnob = par("lnob", [DH, 1])
    odiags = par("odiags", [DH, 9 * DH], dt.bfloat16)
    opw = par("opw", [DH, DIM], dt.bfloat16)
    ln2w = par("ln2w", [1, DIM])
    ln2b = par("ln2b", [1, DIM])
    o64hd = par("o64h", [DH, 1], dt.float32r)
    zpad = par("zpad", [DIM, G], dt.float32r)
    onesrd = par("onesr", [1, DH], dt.float32r)
    out_ext = nc.declare_dram_parameter("out", [N // 4, DIM], dt.float32, isOutput=True)

    rs_in = nc.dram_tensor("rs_in", [N, DIM], dt.float32)
    rs_out = nc.dram_tensor("rs_out", [N // 4, DIM], dt.float32)

    with (
        nc.allow_low_precision(reason="float32r/bf16 compute by design"),
        tile.TileContext(nc) as tc,
        tc.tile_pool(name="main", bufs=1) as main,
        tc.tile_pool(name="tmp2", bufs=2) as tmp2,
    ):
        # ---- persistent SBUF tiles ----
        QL = main.tile([128, N], dt.float32r)  # LN'd q, duplicated on both halves
        KL = main.tile([128, N], dt.float32r)
        VT = main.tile([DH, N], dt.bfloat16)  # v^T for the skip connection
        V = main.tile([128, NCH, DH + 1], dt.bfloat16)
        SC = main.tile([1, 2 * N], dt.float32)  # mu | E2 (E2 becomes var/rs)
        SCB = main.tile([1, 2 * N], dt.float32r)  # rs | mu*rs (matmul-ready)
        Og = main.tile([DH, G, G], dt.bfloat16)  # padded out-LN grid
        o64h = main.tile([DH, 1], dt.float32r)
        nc.sync.dma_start(out=o64h, in_=o64hd[:, :])
        lnqb_t = main.tile([DH, 1], dt.float32)
        lnkb_t = main.tile([DH, 1], dt.float32)
        lnob_t = main.tile([DH, 1], dt.float32)
        nc.sync.dma_start(out=lnqb_t, in_=lnqb[:, :])
        nc.sync.dma_start(out=lnkb_t, in_=lnkb[:, :])
        nc.sync.dma_start(out=lnob_t, in_=lnob[:, :])
        lnqw_t = main.tile([1, DH], dt.float32r)
        lnkw_t = main.tile([1, DH], dt.float32r)
        lnow_t = main.tile([1, DH], dt.float32r)
        nc.sync.dma_start(out=lnqw_t, in_=lnqw[:, :])
        nc.sync.dma_start(out=lnkw_t, in_=lnkw[:, :])
        nc.sync.dma_start(out=lnow_t, in_=lnow[:, :])
        onesr = main.tile([1, DH], dt.float32r)
        nc.sync.dma_start(out=onesr, in_=onesrd[:, :])
        epsP = main.tile([128, 1], dt.float32)
        nc.vector.memset(epsP, EPS)
        nc.vector.memset(V, 1.0)
        nc.vector.memset(Og, 0.0)

        def stats_mms(psp, src_ap, c):
            """mu and E[x^2] rows for a [64, IC] chunk into SC columns c."""
            sq = tmp2.tile([DH, IC], dt.float32r, tag="sq")
            nc.vector.tensor_mul(out=sq, in0=src_ap, in1=src_ap)
            smu = psp.tile([1, IC], dt.float32, tag="smu", bufs=1)
            se2 = psp.tile([1, IC], dt.float32, tag="se2", bufs=1)
            nc.tensor.matmul(smu, o64h, src_ap, start=True, stop=True)
            nc.tensor.matmul(se2, o64h, sq, start=True, stop=True)
            nc.vector.tensor_copy(out=SC[:, c * IC : (c + 1) * IC], in_=smu)
            nc.vector.tensor_copy(out=SC[:, N + c * IC : N + (c + 1) * IC], in_=se2)

        def ln_chain(lo, hi):
            """SC mu/E2 -> SCB rs / mu*rs over columns [lo, hi)."""
            mu = SC[:, lo:hi]
            e2 = SC[:, N + lo : N + hi]
            mrs = SCB[:, N + lo : N + hi]
            nc.vector.scalar_tensor_tensor(
                out=mrs, in0=mu, scalar=-1.0, in1=mu, op0=OP.mult, op1=OP.mult
            )
            nc.vector.tensor_add(out=e2, in0=e2, in1=mrs)
            nc.scalar.activation(out=e2, in_=e2, func=AF.Sqrt, bias=epsP[0:1, :])
            nc.vector.reciprocal_approx_fast(out=e2, in_=e2)
            nc.vector.tensor_copy(out=SCB[:, lo:hi], in_=e2)
            nc.vector.tensor_mul(out=mrs, in0=mu, in1=e2)

        def ln_bcast_apply(psp, w_row, b_t, src_ap, dst, c, dst_is_grid=False, bc_bufs=2):
            """dst chunk c = (src*rs - mu*rs)*w + b via two K=1 bcasts."""
            bcA = psp.tile([DH, IC], dt.float32, tag="bc", bufs=bc_bufs)
            nc.tensor.matmul(
                bcA, w_row, SCB[:, c * IC : (c + 1) * IC], start=True, stop=True
            )
            bcB = psp.tile([DH, IC], dt.float32, tag="bc", bufs=bc_bufs)
            nc.tensor.matmul(
                bcB, w_row, SCB[:, N + c * IC : N + (c + 1) * IC], start=True, stop=True
            )
            T = tmp2.tile([DH, IC], dt.float32, tag="T")
            nc.vector.tensor_mul(out=T, in0=src_ap, in1=bcA)
            if dst_is_grid:
                r0 = c * 8
                nc.vector.scalar_tensor_tensor(
                    out=dst[:, 1 + r0 : 9 + r0, 1 : 1 + Ws],
                    in0=T.rearrange("p (a b) -> p a b", b=Ws),
                    scalar=b_t,
                    in1=bcB.rearrange("p (a b) -> p a b", b=Ws),
                    op0=OP.add,
                    op1=OP.subtract,
                )
            else:
                nc.vector.scalar_tensor_tensor(
                    out=dst[0:DH, c * IC : (c + 1) * IC],
                    in0=T,
                    scalar=b_t,
                    in1=bcB,
                    op0=OP.add,
                    op1=OP.subtract,
                )

        # ============ Stage A1: pos + depthwise ============
        with tc.tile_pool(name="stageA", bufs=1) as pA:
            psA1cm = tc.tile_pool(name="psA1", bufs=2, space="PSUM")
            psA1 = psA1cm.__enter__()
            Xg = pA.tile([DIM, G, G], dt.float32r)
            nc.sync.dma_start(out=Xg[:, 0:1, :], in_=zpad[:, :].unsqueeze(1))
            nc.sync.dma_start(out=Xg[:, G - 1 : G, :], in_=zpad[:, :].unsqueeze(1))
            nc.sync.dma_start(
                out=Xg[:, 1 : G - 1, 0:1], in_=zpad[:, 0 : G - 2].unsqueeze(2)
            )
            nc.sync.dma_start(
                out=Xg[:, 1 : G - 1, G - 1 : G], in_=zpad[:, 0 : G - 2].unsqueeze(2)
            )
            for q in range(4):
                nc.sync.dma_start(
                    out=Xg[:, 1 + 16 * q : 1 + 16 * (q + 1), 1 : 1 + Ws],
                    in_=x[:, 1024 * q : 1024 * (q + 1)]
                    .bitcast(dt.float32r)
                    .rearrange("p (a b) -> p a b", b=Ws),
                )
            pA0 = tc.tile_pool(name="pA0", bufs=1)
            pA0p = pA0.__enter__()
            msh = pA0p.tile([3, N], dt.bfloat16)
            nc.sync.dma_start(out=msh, in_=meshb[:, :])
            pwt = pA0p.tile([3, DIM], dt.bfloat16)
            nc.sync.dma_start(out=pwt, in_=pewT[:, :])
            pwq_t = pA.tile([DIM, DH], dt.float32r)
            pwk_t = pA.tile([DIM, DH], dt.float32r)
            pwv_t = pA.tile([DIM, DH], dt.float32r)
            nc.sync.dma_start(out=pwq_t, in_=pwq[:, :])
            nc.sync.dma_start(out=pwk_t, in_=pwk[:, :])
            nc.sync.dma_start(out=pwv_t, in_=pwv[:, :])

            # pos encoding into the guard interior
            for c in range(NIC):
                pos_ps = psA1.tile([DIM, IC], dt.float32, tag="pos")
                nc.tensor.matmul(
                    pos_ps, pwt, msh[:, c * IC : (c + 1) * IC], start=True, stop=True
                )
                r0 = c * 8
                view = Xg[:, 1 + r0 : 9 + r0, 1 : 1 + Ws]
                nc.vector.tensor_add(
                    out=view, in0=view, in1=pos_ps.rearrange("p (a b) -> p a b", b=Ws)
                )

            pA0.__exit__(None, None, None)
            # depthwise 3x3 via 9 accumulated diag matmuls
            qdg = pA.tile([DIM, 9, DIM], dt.float32r)
            nc.sync.dma_start(
                out=qdg, in_=qdiags[:, :].rearrange("p (t c) -> p t c", t=9)
            )
            Yr = pA.tile([DIM, N], dt.float32r)
            for c in range(NIC):
                dwp = psA1.tile([DIM, IC], dt.float32, tag="dw")
                r0 = c * 8
                t = 0
                for di in range(3):
                    for dj in range(3):
                        nc.tensor.matmul(
                            dwp,
                            qdg[:, t, :],
                            Xg[:, r0 + di : r0 + di + 8, dj : dj + Ws],
                            start=(t == 0),
                            stop=(t == 8),
                        )
                        t += 1
                nc.vector.tensor_copy(out=Yr[:, c * IC : (c + 1) * IC], in_=dwp)

            psA1cm.__exit__(None, None, None)
            # ============ Stage A2: pointwise + q/k LN + v ============
            with tc.tile_pool(name="psA2", bufs=1, space="PSUM") as psA2:
                QRW = pA.tile([DH, N], dt.float32r)
                KRAW = pA.tile([DH, N], dt.float32r)
                for c in range(NIC):
                    qp = psA2.tile([DH, IC], dt.float32, tag="qp", bufs=2)
                    nc.tensor.matmul(
                        qp, pwq_t, Yr[:, c * IC : (c + 1) * IC], start=True, stop=True
                    )
                    nc.vector.tensor_copy(out=QRW[:, c * IC : (c + 1) * IC], in_=qp)
                for c in range(NIC):
                    stats_mms(psA2, QRW[:, c * IC : (c + 1) * IC], c)
                ln_chain(0, N)  # q chain on DVE/ACT...

                # ...while the PE does k's pointwise
                for c in range(NIC):
                    qp = psA2.tile([DH, IC], dt.float32, tag="qp", bufs=2)
                    nc.tensor.matmul(
                        qp, pwk_t, Yr[:, c * IC : (c + 1) * IC], start=True, stop=True
                    )
                    nc.vector.tensor_copy(out=KRAW[:, c * IC : (c + 1) * IC], in_=qp)

                # q broadcast + apply -> QL lower half, then mirror upward
                for c in range(NIC):
                    ln_bcast_apply(psA2, lnqw_t, lnqb_t, QRW[:, c * IC : (c + 1) * IC], QL, c)
                nc.vector.tensor_copy(out=QL[DH:128, :], in_=QL[0:DH, :])

                for c in range(NIC):
                    stats_mms(psA2, KRAW[:, c * IC : (c + 1) * IC], c)
                ln_chain(0, N)  # k chain...

                # ...while the PE builds v (N-layout) and v^T
                for ch in range(NCH):
                    vp = psA2.tile([128, DH], dt.float32, tag="vp", bufs=2)
                    nc.tensor.matmul(
                        vp,
                        Yr[:, ch * 128 : (ch + 1) * 128],
                        pwv_t,
                        start=True,
                        stop=True,
                    )
                    nc.vector.tensor_copy(out=V[:, ch, 0:DH], in_=vp)
                for c in range(NIC):
                    qp = psA2.tile([DH, IC], dt.float32, tag="qp", bufs=2)
                    nc.tensor.matmul(
                        qp, pwv_t, Yr[:, c * IC : (c + 1) * IC], start=True, stop=True
                    )
                    nc.vector.tensor_copy(out=VT[:, c * IC : (c + 1) * IC], in_=qp)

                for c in range(NIC):
                    ln_bcast_apply(psA2, lnkw_t, lnkb_t, KRAW[:, c * IC : (c + 1) * IC], KL, c)
                nc.vector.tensor_copy(out=KL[DH:128, :], in_=KL[0:DH, :])

        # ============ Stage B: attention with inline out-LN ============
        with tc.tile_pool(name="psB", bufs=1, space="PSUM") as psB, tc.tile_pool(
            name="sbB", bufs=3
        ) as sbB:
            for c in range(NIC):
                avp = psB.tile([DH + 1, IC], dt.float32, tag="avp", bufs=1)
                stgs = {}
                Es = {}

                def issue_st(g, c=c, stgs=stgs):
                    # row-packed pair: j-block 2g on array rows 0-63 and
                    # 2g+1 on rows 64-127, running concurrently
                    stg = psB.tile([128, 2 * IC], dt.float32, tag="stg", bufs=2)
                    j0 = 2 * g * JB
                    nc.tensor.matmul(
                        stg[:, 0:IC],
                        KL[0:DH, j0 : j0 + JB],
                        QL[0:DH, c * IC : (c + 1) * IC],
                        start=True,
                        stop=True,
                    )
                    nc.tensor.matmul(
                        stg[:, IC : 2 * IC],
                        KL[DH:128, j0 + JB : j0 + 2 * JB],
                        QL[DH:128, c * IC : (c + 1) * IC],
                        start=True,
                        stop=True,
                    )
                    stgs[g] = stg

                def issue_exp(g, stgs=stgs, Es=Es):
                    E = sbB.tile([128, 2 * IC], dt.bfloat16, tag="E")
                    nc.scalar.activation(
                        out=E, in_=stgs.pop(g), func=AF.Exp, scale=float(DH**-0.5)
                    )
                    Es[g] = E

                def issue_av(g, c=c, Es=Es, avp=avp):
                    E = Es.pop(g)
                    for t in range(2):
                        jb = 2 * g + t
                        nc.tensor.matmul(
                            avp,
                            V[:, jb, :],
                            E[:, t * IC : (t + 1) * IC],
                            start=(jb == 0),
                            stop=(jb == NJB - 1),
                            skip_group_check=True,
                        )

                NG = NJB // 2
                issue_st(0)
                issue_exp(0)
                for g in range(1, NG):
                    issue_st(g)
                    issue_exp(g)
                    issue_av(g - 1)
                issue_av(NG - 1)

                # inline per-chunk softmax-normalize + skip + out-LN
                REC = sbB.tile([1, IC], dt.float32, tag="REC", bufs=2)
                nc.vector.tensor_copy(out=REC, in_=avp[DH : DH + 1, :])
                nc.vector.reciprocal_approx_fast(out=REC, in_=REC)
                RECB = sbB.tile([1, IC], dt.float32r, tag="RECB", bufs=2)
                nc.vector.tensor_copy(out=RECB, in_=REC)
                bcR = psB.tile([DH, IC], dt.float32, tag="bc", bufs=1)
                nc.tensor.matmul(bcR, onesr, RECB, start=True, stop=True)
                Tn = sbB.tile([DH, IC], dt.float32, tag="Tn", bufs=2)
                nc.vector.tensor_copy(out=Tn, in_=avp[0:DH, :])
                OSc = sbB.tile([DH, IC], dt.float32r, tag="OS", bufs=2)
                nc.vector.tensor_mul(out=OSc, in0=Tn, in1=bcR)
                nc.vector.tensor_add(
                    out=OSc, in0=OSc, in1=VT[:, c * IC : (c + 1) * IC]
                )
                stats_mms(psB, OSc[:, :], c)
                ln_chain(c * IC, (c + 1) * IC)
                ln_bcast_apply(psB, lnow_t, lnob_t, OSc[:, :], Og, c, dst_is_grid=True, bc_bufs=1)

        # ============ Stage C: out depthwise + pointwise partial ============
        with (
            tc.tile_pool(name="stageC", bufs=1) as pC,
            tc.tile_pool(name="psC", bufs=2, space="PSUM") as psC,
        ):
            odg = pC.tile([DH, 9, DH], dt.bfloat16)
            nc.sync.dma_start(
                out=odg, in_=odiags[:, :].rearrange("p (t c) -> p t c", t=9)
            )
            opw_t = pC.tile([DH, DIM], dt.bfloat16)
            nc.sync.dma_start(out=opw_t, in_=opw[:, :])
            DWO = pC.tile([DH, N], dt.bfloat16)
            for c in range(NIC):
                dwp = psC.tile([DH, IC], dt.float32, tag="dw")
                r0 = c * 8
                t = 0
                for di in range(3):
                    for dj in range(3):
                        nc.tensor.matmul(
                            dwp,
                            odg[:, t, :],
                            Og[:, r0 + di : r0 + di + 8, dj : dj + Ws],
                            start=(t == 0),
                            stop=(t == 8),
                        )
                        t += 1
                nc.vector.tensor_copy(out=DWO[:, c * IC : (c + 1) * IC], in_=dwp)
            for ch in range(NCH):
                pp = psC.tile([128, DIM], dt.float32, tag="pp")
                nc.tensor.matmul(
                    pp, DWO[:, ch * 128 : (ch + 1) * 128], opw_t, start=True, stop=True
                )
                PP = tmp2.tile([128, DIM], dt.float32, tag="PP")
                nc.vector.tensor_copy(out=PP, in_=pp)
                nc.sync.dma_start(out=rs_in[ch * 128 : (ch + 1) * 128, :], in_=PP)

        # ============ Stage D: ReduceScatter + LayerNorm2d ============
        nc.gpsimd.collective_compute(
            "ReduceScatter",
            OP.add,
            replica_groups=[[0, 1, 2, 3], [4, 5, 6, 7]],
            ins=[rs_in[:, :]],
            outs=[rs_out[:, :]],
        )
        with tc.tile_pool(name="stageD", bufs=2) as pD:
            w_b = pD.tile([128, DIM], dt.float32, bufs=1)
            b_b = pD.tile([128, DIM], dt.float32, bufs=1)
            nc.sync.dma_start(out=w_b, in_=ln2w[:, :].to_broadcast([128, DIM]))
            nc.sync.dma_start(out=b_b, in_=ln2b[:, :].to_broadcast([128, DIM]))
            for tkn in range(8):
                R = pD.tile([128, DIM], dt.float32, tag="R")
                nc.sync.dma_start(out=R, in_=rs_out[tkn * 128 : (tkn + 1) * 128, :])
                st = pD.tile([128, 6], dt.float32, tag="st")
                nc.vector.bn_stats(out=st, in_=R)
                mv = pD.tile([128, 2], dt.float32, tag="mv")
                nc.vector.bn_aggr(out=mv, in_=st)
                sd = pD.tile([128, 1], dt.float32, tag="sd")
                nc.scalar.activation(out=sd, in_=mv[:, 1:2], func=AF.Sqrt, bias=epsP)
                nc.vector.reciprocal(out=sd, in_=sd)
                nc.vector.tensor_scalar(
                    out=R,
                    in0=R,
                    scalar1=mv[:, 0:1],
                    scalar2=sd,
                    op0=OP.subtract,
                    op1=OP.mult,
                )
                R2 = pD.tile([128, DIM], dt.float32, tag="R2")
                nc.vector.tensor_mul(out=R2, in0=R, in1=w_b)
                nc.vector.tensor_add(out=R2, in0=R2, in1=b_b)
                nc.sync.dma_start(out=out_ext[tkn * 128 : (tkn + 1) * 128, :], in_=R2)

    return nc


_cached = {}


def _get_nc():
    if "nc" not in _cached:
        nc = _build()
        nc.finalize()
        _cached["nc"] = nc
    return _cached["nc"]


def _make_in_maps(inputs):
    import ml_dtypes

    x = np.asarray(inputs["x"], np.float32)
    pe_w = np.asarray(inputs["pe_w"], np.float32)
    pe_b = np.asarray(inputs["pe_b"], np.float32)
    qkv_dw = np.asarray(inputs["qkv_dw"], np.float32)
    qkv_pw = np.asarray(inputs["qkv_pw"], np.float32)
    out_dw = np.asarray(inputs["out_dw"], np.float32)
    out_pw = np.asarray(inputs["out_pw"], np.float32)
    nq_w, nq_b = np.asarray(inputs["nq_w"], np.float32), np.asarray(
        inputs["nq_b"], np.float32
    )
    nk_w, nk_b = np.asarray(inputs["nk_w"], np.float32), np.asarray(
        inputs["nk_b"], np.float32
    )
    no_w, no_b = np.asarray(inputs["no_w"], np.float32), np.asarray(
        inputs["no_b"], np.float32
    )
    ln_w, ln_b = np.asarray(inputs["ln_w"], np.float32), np.asarray(
        inputs["ln_b"], np.float32
    )

    gx = np.linspace(0.0, 1.0, Hs, dtype=np.float32)
    gy = np.linspace(0.0, 1.0, Ws, dtype=np.float32)
    meshb = np.stack(
        [np.repeat(gx, Ws), np.tile(gy, Hs), np.ones(N, np.float32)]
    ).astype(ml_dtypes.bfloat16)
    pewT = np.stack([pe_w[:, 0], pe_w[:, 1], pe_b]).astype(ml_dtypes.bfloat16)

    idx = np.arange(DH)
    in_maps = []
    for c in range(8):
        b, h = c // 4, c % 4
        rows = h + HEADS * idx
        qdiags = np.zeros((DIM, 9, DIM), np.float32)
        taps = qkv_dw.reshape(DIM, 9)
        for t in range(9):
            qdiags[np.arange(DIM), t, np.arange(DIM)] = taps[:, t]
        odiags = np.zeros((DH, 9, DH), np.float32)
        otaps = out_dw[rows].reshape(DH, 9)
        for t in range(9):
            odiags[idx, t, idx] = otaps[:, t]
        m = {
            "x": np.ascontiguousarray(x[b].reshape(DIM, N)),
            "meshb": meshb,
            "pewT": pewT,
            "qdiags": np.ascontiguousarray(qdiags.reshape(DIM, 9 * DIM)),
            "pwq": np.ascontiguousarray(qkv_pw[rows, :].T),
            "pwk": np.ascontiguousarray(qkv_pw[DIM * 2 + rows, :].T),
            "pwv": np.ascontiguousarray(qkv_pw[DIM * 4 + rows, :].T),
            "lnqw": np.ascontiguousarray(nq_w[h][None, :]),
            "lnqb": np.ascontiguousarray(nq_b[h][:, None]),
            "lnkw": np.ascontiguousarray(nk_w[h][None, :]),
            "lnkb": np.ascontiguousarray(nk_b[h][:, None]),
            "lnow": np.ascontiguousarray(no_w[h][None, :]),
            "lnob": np.ascontiguousarray(no_b[h][:, None]),
            "odiags": np.ascontiguousarray(odiags.reshape(DH, 9 * DH)).astype(
                ml_dtypes.bfloat16
            ),
            "opw": np.ascontiguousarray(out_pw[:, rows].T).astype(ml_dtypes.bfloat16),
            "ln2w": np.ascontiguousarray(ln_w[None, :]),
            "ln2b": np.ascontiguousarray(ln_b[None, :]),
            "o64h": np.full((DH, 1), 1.0 / DH, np.float32),
            "zpad": np.zeros((DIM, G), np.float32),
            "onesr": np.ones((1, DH), np.float32),
        }
        in_maps.append(m)
    return in_maps


def run_on_device(inputs, **kw):
    nc = _get_nc()
    in_maps = _make_in_maps(inputs)
    res = run_bass_kernel_spmd(nc, in_maps, core_ids=list(range(8)), **kw)
    out = np.zeros((B, DIM, N), np.float32)
    for c in range(8):
        b, h = c // 4, c % 4
        out[b][:, h * (N // 4) : (h + 1) * (N // 4)] = res.results[c]["out"].T
    return out.reshape(B, DIM, Hs, Ws), res


def kernel(**inputs):
    out, _ = run_on_device(inputs)
    return out


# revision 24
# speedup vs baseline: 1.3334x; 1.0473x over previous
"""Trainium2 Bass kernel for nn_Attention_19404662243470.

Sharding: 8 cores = (batch 2) x (heads 4). Each core computes the full
attention pipeline for its (b, h) pair in transposed layout [d, n]; the
final pointwise conv partials are ReduceScattered within each batch's
4-core group, and LayerNorm2d runs on each core's position shard.

Layout notes:
 - q/k/v come out of the pointwise conv directly as [d, n] ("T layout"),
   which is exactly the operand layout the S^T = K Q^T matmul needs.
 - softmax runs without max-subtraction (logits are bounded ~|5|); the
   denominator falls out of the AV matmul via an appended ones-row in V.
 - per-head LN over d (the partition dim) uses ones-matmuls for the
   stats and K=1 broadcast matmuls to spread per-column scalars.
 - S^T pairs are row-packed onto the two halves of the PE array
   (contraction is only 64 deep), doubling S^T throughput.
 - long PE idle gaps are avoided (HAM throttles the PE clock to 1.2 GHz
   after ~3.4us of idle and has been seen never to recover): LN scalar
   chains are overlapped with independent matmul work, and the out-LN
   is folded into the per-chunk attention loop.
"""

import numpy as np

import concourse.bass as bass
import concourse.tile as tile
from concourse import bacc, mybir
from concourse.bass_utils import run_bass_kernel_spmd

dt = mybir.dt
AF = mybir.ActivationFunctionType
OP = mybir.AluOpType

B, DIM, Hs, Ws = 2, 128, 64, 64
HEADS, DH = 4, 64
N = Hs * Ws  # 4096
EPS = 1e-6
IC = 512  # i-chunk width
NIC = N // IC  # 8
JB = 128  # j-block
NJB = N // JB  # 32
NCH = N // 128  # 32
G = Hs + 2  # 66 padded grid


def _build():
    nc = bacc.Bacc()

    def par(name, shape, dtyp=dt.float32):
        return nc.declare_dram_parameter(name, list(shape), dtyp, isOutput=False)

    x = par("x", [DIM, N])
    meshb = par("meshb", [3, N], dt.bfloat16)
    pewT = par("pewT", [3, DIM], dt.bfloat16)
    qdiags = par("qdiags", [DIM, 9 * DIM], dt.float32r)
    pwq = par("pwq", [DIM, DH], dt.float32r)
    pwk = par("pwk", [DIM, DH], dt.float32r)
    pwv = par("pwv", [DIM, DH], dt.float32r)
    lnqw = par("lnqw", [1, DH], dt.float32r)
    lnqb = par("lnqb", [DH, 1])
    lnkw = par("lnkw", [1, DH], dt.float32r)
    lnkb = par("lnkb", [DH, 1])
    lnow = par("lnow", [1, DH], dt.float32r)
    lnob = par("lnob", [DH, 1])
    odiags = par("odiags", [DH, 9 * DH], dt.bfloat16)
    opw = par("opw", [DH, DIM], dt.bfloat16)
    ln2w = par("ln2w", [1, DIM])
    ln2b = par("ln2b", [1, DIM])
    o64hd = par("o64h", [DH, 1], dt.float32r)
    zpad = par("zpad", [DIM, G], dt.float32r)
    onesrd = par("onesr", [1, DH], dt.float32r)
    out_ext = nc.declare_dram_parameter("out", [N // 4, DIM], dt.float32, isOutput=True)

    rs_in = nc.dram_tensor("rs_in", [N, DIM], dt.float32)
    rs_out = nc.dram_tensor("rs_out", [N // 4, DIM], dt.float32)

    with (
        nc.allow_low_precision(reason="float32r/bf16 compute by design"),
        tile.TileContext(nc) as tc,
        tc.tile_pool(name="main", bufs=1) as main,
        tc.tile_pool(name="tmp2", bufs=2) as tmp2,
    ):
        # ---- persistent SBUF tiles ----
        QL = main.tile([128, N], dt.float32r)  # LN'd q, duplicated on both halves
        KL = main.tile([128, N], dt.float32r)
        VT = main.tile([DH, N], dt.bfloat16)  # v^T for the skip connection
        V = main.tile([128, NCH, DH + 1], dt.bfloat16)
        SC = main.tile([1, 2 * N], dt.float32)  # mu | E2 (E2 becomes var/rs)
        SCB = main.tile([1, 2 * N], dt.float32r)  # rs | mu*rs (matmul-ready)
        Og = main.tile([DH, G, G], dt.bfloat16)  # padded out-LN grid
        o64h = main.tile([DH, 1], dt.float32r)
        nc.sync.dma_start(out=o64h, in_=o64hd[:, :])
        lnqb_t = main.tile([DH, 1], dt.float32)
        lnkb_t = main.tile([DH, 1], dt.float32)
        lnob_t = main.tile([DH, 1], dt.float32)
        nc.sync.dma_start(out=lnqb_t, in_=lnqb[:, :])
        nc.sync.dma_start(out=lnkb_t, in_=lnkb[:, :])
        nc.sync.dma_start(out=lnob_t, in_=lnob[:, :])
        lnqw_t = main.tile([1, DH], dt.float32r)
        lnkw_t = main.tile([1, DH], dt.float32r)
        lnow_t = main.tile([1, DH], dt.float32r)
        nc.sync.dma_start(out=lnqw_t, in_=lnqw[:, :])
        nc.sync.dma_start(out=lnkw_t, in_=lnkw[:, :])
        nc.sync.dma_start(out=lnow_t, in_=lnow[:, :])
        onesr = main.tile([1, DH], dt.float32r)
        nc.sync.dma_start(out=onesr, in_=onesrd[:, :])
        epsP = main.tile([128, 1], dt.float32)
        nc.vector.memset(epsP, EPS)
        nc.vector.memset(V, 1.0)
        nc.vector.memset(Og, 0.0)

        def stats_mms(psp, src_ap, c):
            """mu and E[x^2] rows for a [64, IC] chunk into SC columns c."""
            sq = tmp2.tile([DH, IC], dt.float32r, tag="sq")
            nc.vector.tensor_mul(out=sq, in0=src_ap, in1=src_ap)
            smu = psp.tile([1, IC], dt.float32, tag="smu", bufs=1)
            se2 = psp.tile([1, IC], dt.float32, tag="se2", bufs=1)
            nc.tensor.matmul(smu, o64h, src_ap, start=True, stop=True)
            nc.tensor.matmul(se2, o64h, sq, start=True, stop=True)
            nc.vector.tensor_copy(out=SC[:, c * IC : (c + 1) * IC], in_=smu)
            nc.vector.tensor_copy(out=SC[:, N + c * IC : N + (c + 1) * IC], in_=se2)

        def ln_chain(lo, hi):
            """SC mu/E2 -> SCB rs / mu*rs over columns [lo, hi)."""
            mu = SC[:, lo:hi]
            e2 = SC[:, N + lo : N + hi]
            mrs = SCB[:, N + lo : N + hi]
            nc.vector.scalar_tensor_tensor(
                out=mrs, in0=mu, scalar=-1.0, in1=mu, op0=OP.mult, op1=OP.mult
            )
            nc.vector.tensor_add(out=e2, in0=e2, in1=mrs)
            nc.scalar.activation(out=e2, in_=e2, func=AF.Sqrt, bias=epsP[0:1, :])
            nc.vector.reciprocal_approx_fast(out=e2, in_=e2)
            nc.vector.tensor_copy(out=SCB[:, lo:hi], in_=e2)
            nc.vector.tensor_mul(out=mrs, in0=mu, in1=e2)

        def ln_bcast_apply(psp, w_row, b_t, src_ap, dst, c, dst_is_grid=False, bc_bufs=2):
            """dst chunk c = (src*rs - mu*rs)*w + b via two K=1 bcasts."""
            bcA = psp.tile([DH, IC], dt.float32, tag="bc", bufs=bc_bufs)
            nc.tensor.matmul(
                bcA, w_row, SCB[:, c * IC : (c + 1) * IC], start=True, stop=True
            )
            bcB = psp.tile([DH, IC], dt.float32, tag="bc", bufs=bc_bufs)
            nc.tensor.matmul(
                bcB, w_row, SCB[:, N + c * IC : N + (c + 1) * IC], start=True, stop=True
            )
            T = tmp2.tile([DH, IC], dt.float32, tag="T")
            nc.vector.tensor_mul(out=T, in0=src_ap, in1=bcA)
            if dst_is_grid:
                r0 = c * 8
                nc.vector.scalar_tensor_tensor(
                    out=dst[:, 1 + r0 : 9 + r0, 1 : 1 + Ws],
                    in0=T.rearrange("p (a b) -> p a b", b=Ws),
                    scalar=b_t,
                    in1=bcB.rearrange("p (a b) -> p a b", b=Ws),
                    op0=OP.add,
                    op1=OP.subtract,
                )
            else:
                nc.vector.scalar_tensor_tensor(
                    out=dst[0:DH, c * IC : (c + 1) * IC],
                    in0=T,
                    scalar=b_t,
                    in1=bcB,
                    op0=OP.add,
                    op1=OP.subtract,
                )

        # ============ Stage A1: pos + depthwise ============
        with tc.tile_pool(name="stageA", bufs=1) as pA:
            psA1cm = tc.tile_pool(name="psA1", bufs=2, space="PSUM")
            psA1 = psA1cm.__enter__()
            Xg = pA.tile([DIM, G, G], dt.float32r)
            pA0 = tc.tile_pool(name="pA0", bufs=1)
            pA0p = pA0.__enter__()
            msh = pA0p.tile([3, N], dt.bfloat16)
            nc.scalar.dma_start(out=msh, in_=meshb[:, :])
            pwt = pA0p.tile([3, DIM], dt.bfloat16)
            nc.scalar.dma_start(out=pwt, in_=pewT[:, :])
            nc.sync.dma_start(out=Xg[:, 0:1, :], in_=zpad[:, :].unsqueeze(1))
            nc.sync.dma_start(out=Xg[:, G - 1 : G, :], in_=zpad[:, :].unsqueeze(1))
            nc.scalar.dma_start(
                out=Xg[:, 1 : G - 1, 0:1], in_=zpad[:, 0 : G - 2].unsqueeze(2)
            )
            nc.scalar.dma_start(
                out=Xg[:, 1 : G - 1, G - 1 : G], in_=zpad[:, 0 : G - 2].unsqueeze(2)
            )
            for q in range(4):
                eng = nc.sync if q % 2 == 0 else nc.scalar
                eng.dma_start(
                    out=Xg[:, 1 + 16 * q : 1 + 16 * (q + 1), 1 : 1 + Ws],
                    in_=x[:, 1024 * q : 1024 * (q + 1)]
                    .bitcast(dt.float32r)
                    .rearrange("p (a b) -> p a b", b=Ws),
                )
            pwq_t = pA.tile([DIM, DH], dt.float32r)
            pwk_t = pA.tile([DIM, DH], dt.float32r)
            pwv_t = pA.tile([DIM, DH], dt.float32r)
            nc.sync.dma_start(out=pwq_t, in_=pwq[:, :])
            nc.sync.dma_start(out=pwk_t, in_=pwk[:, :])
            nc.sync.dma_start(out=pwv_t, in_=pwv[:, :])

            # pos encoding into the guard interior
            for c in range(NIC):
                pos_ps = psA1.tile([DIM, IC], dt.float32, tag="pos")
                nc.tensor.matmul(
                    pos_ps, pwt, msh[:, c * IC : (c + 1) * IC], start=True, stop=True
                )
                r0 = c * 8
                view = Xg[:, 1 + r0 : 9 + r0, 1 : 1 + Ws]
                nc.vector.tensor_add(
                    out=view, in0=view, in1=pos_ps.rearrange("p (a b) -> p a b", b=Ws)
                )

            pA0.__exit__(None, None, None)
            # depthwise 3x3 via 9 accumulated diag matmuls
            qdg = pA.tile([DIM, 9, DIM], dt.float32r)
            nc.scalar.dma_start(
                out=qdg, in_=qdiags[:, :].rearrange("p (t c) -> p t c", t=9)
            )
            Yr = pA.tile([DIM, N], dt.float32r)
            for c in range(NIC):
                dwp = psA1.tile([DIM, IC], dt.float32, tag="dw")
                r0 = c * 8
                t = 0
                for di in range(3):
                    for dj in range(3):
                        nc.tensor.matmul(
                            dwp,
                            qdg[:, t, :],
                            Xg[:, r0 + di : r0 + di + 8, dj : dj + Ws],
                            start=(t == 0),
                            stop=(t == 8),
                        )
                        t += 1
                nc.vector.tensor_copy(out=Yr[:, c * IC : (c + 1) * IC], in_=dwp)

            psA1cm.__exit__(None, None, None)
            # ============ Stage A2: pointwise + q/k LN + v ============
            with tc.tile_pool(name="psA2", bufs=1, space="PSUM") as psA2:
                QRW = pA.tile([DH, N], dt.float32r)
                KRAW = pA.tile([DH, N], dt.float32r)
                for c in range(NIC):
                    qp = psA2.tile([DH, IC], dt.float32, tag="qp", bufs=2)
                    nc.tensor.matmul(
                        qp, pwq_t, Yr[:, c * IC : (c + 1) * IC], start=True, stop=True
                    )
                    nc.vector.tensor_copy(out=QRW[:, c * IC : (c + 1) * IC], in_=qp)
                for c in range(NIC):
                    stats_mms(psA2, QRW[:, c * IC : (c + 1) * IC], c)
                # q chain on DVE/ACT runs while the PE does k's pointwise
                ln_chain(0, N)
                for c in range(NIC):
                    qp = psA2.tile([DH, IC], dt.float32, tag="qp", bufs=2)
                    nc.tensor.matmul(
                        qp, pwk_t, Yr[:, c * IC : (c + 1) * IC], start=True, stop=True
                    )
                    nc.vector.tensor_copy(out=KRAW[:, c * IC : (c + 1) * IC], in_=qp)

                # q broadcast + apply -> QL lower half, then mirror upward
                for c in range(NIC):
                    ln_bcast_apply(psA2, lnqw_t, lnqb_t, QRW[:, c * IC : (c + 1) * IC], QL, c)
                nc.vector.tensor_copy(out=QL[DH:128, :], in_=QL[0:DH, :])

                for c in range(NIC):
                    stats_mms(psA2, KRAW[:, c * IC : (c + 1) * IC], c)
                # k chain runs while the PE builds v (N-layout) and v^T
                ln_chain(0, N)
                for ch in range(NCH):
                    vp = psA2.tile([128, DH], dt.float32, tag="vp", bufs=2)
                    nc.tensor.matmul(
                        vp,
                        Yr[:, ch * 128 : (ch + 1) * 128],
                        pwv_t,
                        start=True,
                        stop=True,
                    )
                    nc.vector.tensor_copy(out=V[:, ch, 0:DH], in_=vp)
                for c in range(NIC):
                    qp = psA2.tile([DH, IC], dt.float32, tag="qp", bufs=2)
                    nc.tensor.matmul(
                        qp, pwv_t, Yr[:, c * IC : (c + 1) * IC], start=True, stop=True
                    )
                    nc.vector.tensor_copy(out=VT[:, c * IC : (c + 1) * IC], in_=qp)

                for c in range(NIC):
                    ln_bcast_apply(psA2, lnkw_t, lnkb_t, KRAW[:, c * IC : (c + 1) * IC], KL, c)
                nc.vector.tensor_copy(out=KL[DH:128, :], in_=KL[0:DH, :])

        # ============ Stage B: attention with inline out-LN ============
        with tc.tile_pool(name="psB", bufs=1, space="PSUM") as psB, tc.tile_pool(
            name="sbB", bufs=3
        ) as sbB:
            NG = NJB // 2
            pending_tail = []

            def attention_block(c):
                avp = psB.tile([DH + 1, IC], dt.float32, tag="avp", bufs=1)
                stgs = {}
                Es = {}

                def issue_st(g):
                    stg = psB.tile([128, 2 * IC], dt.float32, tag="stg", bufs=2)
                    j0 = 2 * g * JB
                    nc.tensor.matmul(
                        stg[:, 0:IC],
                        KL[0:DH, j0 : j0 + JB],
                        QL[0:DH, c * IC : (c + 1) * IC],
                        start=True,
                        stop=True,
                    )
                    nc.tensor.matmul(
                        stg[:, IC : 2 * IC],
                        KL[DH:128, j0 + JB : j0 + 2 * JB],
                        QL[DH:128, c * IC : (c + 1) * IC],
                        start=True,
                        stop=True,
                    )
                    stgs[g] = stg

                def issue_exp(g):
                    E = sbB.tile([128, 2 * IC], dt.bfloat16, tag="E")
                    nc.scalar.activation(
                        out=E, in_=stgs.pop(g), func=AF.Exp, scale=float(DH**-0.5)
                    )
                    Es[g] = E

                def issue_av(g):
                    E = Es.pop(g)
                    for t in range(2):
                        jb = 2 * g + t
                        nc.tensor.matmul(
                            avp,
                            V[:, jb, :],
                            E[:, t * IC : (t + 1) * IC],
                            start=(jb == 0),
                            stop=(jb == NJB - 1),
                            skip_group_check=True,
                        )

                issue_st(0)
                issue_exp(0)
                for g in range(1, NG):
                    issue_st(g)
                    issue_exp(g)
                    issue_av(g - 1)
                issue_av(NG - 1)

                # free avp quickly: park the numerator and denominator
                REC = sbB.tile([1, IC], dt.float32, tag="REC", bufs=2)
                nc.vector.tensor_copy(out=REC, in_=avp[DH : DH + 1, :])
                Tn = sbB.tile([DH, IC], dt.float32, tag="Tn", bufs=2)
                nc.vector.tensor_copy(out=Tn, in_=avp[0:DH, :])
                return REC, Tn

            def tail_block(c, REC, Tn):
                # DVE/ACT chain + the few tail matmuls for chunk c; issued
                # after the NEXT chunk's attention matmuls so the PE stream
                # never waits on the chain.
                nc.vector.reciprocal_approx_fast(out=REC, in_=REC)
                RECB = sbB.tile([1, IC], dt.float32r, tag="RECB", bufs=2)
                nc.vector.tensor_copy(out=RECB, in_=REC)
                bcR = psB.tile([DH, IC], dt.float32, tag="bc", bufs=1)
                nc.tensor.matmul(bcR, onesr, RECB, start=True, stop=True)
                OSc = sbB.tile([DH, IC], dt.float32r, tag="OS", bufs=2)
                nc.vector.tensor_mul(out=OSc, in0=Tn, in1=bcR)
                nc.vector.tensor_add(
                    out=OSc, in0=OSc, in1=VT[:, c * IC : (c + 1) * IC]
                )
                stats_mms(psB, OSc[:, :], c)
                ln_chain(c * IC, (c + 1) * IC)
                ln_bcast_apply(
                    psB, lnow_t, lnob_t, OSc[:, :], Og, c, dst_is_grid=True, bc_bufs=1
                )

            for c in range(NIC):
                rec_tn = attention_block(c)
                if pending_tail:
                    tail_block(*pending_tail.pop())
                pending_tail.append((c, *rec_tn))
            tail_block(*pending_tail.pop())

        # ============ Stage C: out depthwise + pointwise partial ============
        with (
            tc.tile_pool(name="stageC", bufs=1) as pC,
            tc.tile_pool(name="psC", bufs=2, space="PSUM") as psC,
        ):
            odg = pC.tile([DH, 9, DH], dt.bfloat16)
            nc.sync.dma_start(
                out=odg, in_=odiags[:, :].rearrange("p (t c) -> p t c", t=9)
            )
            opw_t = pC.tile([DH, DIM], dt.bfloat16)
            nc.sync.dma_start(out=opw_t, in_=opw[:, :])
            DWO = pC.tile([DH, N], dt.bfloat16)
            for c in range(NIC):
                dwp = psC.tile([DH, IC], dt.float32, tag="dw")
                r0 = c * 8
                t = 0
                for di in range(3):
                    for dj in range(3):
                        nc.tensor.matmul(
                            dwp,
                            odg[:, t, :],
                            Og[:, r0 + di : r0 + di + 8, dj : dj + Ws],
                            start=(t == 0),
                            stop=(t == 8),
                        )
                        t += 1
                nc.vector.tensor_copy(out=DWO[:, c * IC : (c + 1) * IC], in_=dwp)
            for ch in range(NCH):
                pp = psC.tile([128, DIM], dt.float32, tag="pp")
                nc.tensor.matmul(
                    pp, DWO[:, ch * 128 : (ch + 1) * 128], opw_t, start=True, stop=True
                )
                PP = tmp2.tile([128, DIM], dt.float32, tag="PP")
                nc.vector.tensor_copy(out=PP, in_=pp)
                nc.sync.dma_start(out=rs_in[ch * 128 : (ch + 1) * 128, :], in_=PP)

        # ============ Stage D: ReduceScatter + LayerNorm2d ============
        nc.gpsimd.collective_compute(
            "ReduceScatter",
            OP.add,
            replica_groups=[[0, 1, 2, 3], [4, 5, 6, 7]],
            ins=[rs_in[:, :]],
            outs=[rs_out[:, :]],
        )
        with tc.tile_pool(name="stageD", bufs=2) as pD:
            w_b = pD.tile([128, DIM], dt.float32, bufs=1)
            b_b = pD.tile([128, DIM], dt.float32, bufs=1)
            nc.sync.dma_start(out=w_b, in_=ln2w[:, :].to_broadcast([128, DIM]))
            nc.sync.dma_start(out=b_b, in_=ln2b[:, :].to_broadcast([128, DIM]))
            for tkn in range(8):
                R = pD.tile([128, DIM], dt.float32, tag="R")
                nc.sync.dma_start(out=R, in_=rs_out[tkn * 128 : (tkn + 1) * 128, :])
                st = pD.tile([128, 6], dt.float32, tag="st")
                nc.vector.bn_stats(out=st, in_=R)
                mv = pD.tile([128, 2], dt.float32, tag="mv")
                nc.vector.bn_aggr(out=mv, in_=st)
                sd = pD.tile([128, 1], dt.float32, tag="sd")
                nc.scalar.activation(out=sd, in_=mv[:, 1:2], func=AF.Sqrt, bias=epsP)
                nc.vector.reciprocal(out=sd, in_=sd)
                nc.vector.tensor_scalar(
                    out=R,
                    in0=R,
                    scalar1=mv[:, 0:1],
                    scalar2=sd,
                    op0=OP.subtract,
                    op1=OP.mult,
                )
                R2 = pD.tile([128, DIM], dt.float32, tag="R2")
                nc.vector.tensor_mul(out=R2, in0=R, in1=w_b)
                nc.vector.tensor_add(out=R2, in0=R2, in1=b_b)
                nc.sync.dma_start(out=out_ext[tkn * 128 : (tkn + 1) * 128, :], in_=R2)

    return nc


_cached = {}


def _get_nc():
    if "nc" not in _cached:
        nc = _build()
        nc.finalize()
        _cached["nc"] = nc
    return _cached["nc"]


def _make_in_maps(inputs):
    import ml_dtypes

    x = np.asarray(inputs["x"], np.float32)
    pe_w = np.asarray(inputs["pe_w"], np.float32)
    pe_b = np.asarray(inputs["pe_b"], np.float32)
    qkv_dw = np.asarray(inputs["qkv_dw"], np.float32)
    qkv_pw = np.asarray(inputs["qkv_pw"], np.float32)
    out_dw = np.asarray(inputs["out_dw"], np.float32)
    out_pw = np.asarray(inputs["out_pw"], np.float32)
    nq_w, nq_b = np.asarray(inputs["nq_w"], np.float32), np.asarray(
        inputs["nq_b"], np.float32
    )
    nk_w, nk_b = np.asarray(inputs["nk_w"], np.float32), np.asarray(
        inputs["nk_b"], np.float32
    )
    no_w, no_b = np.asarray(inputs["no_w"], np.float32), np.asarray(
        inputs["no_b"], np.float32
    )
    ln_w, ln_b = np.asarray(inputs["ln_w"], np.float32), np.asarray(
        inputs["ln_b"], np.float32
    )

    gx = np.linspace(0.0, 1.0, Hs, dtype=np.float32)
    gy = np.linspace(0.0, 1.0, Ws, dtype=np.float32)
    meshb = np.stack(
        [np.repeat(gx, Ws), np.tile(gy, Hs), np.ones(N, np.float32)]
    ).astype(ml_dtypes.bfloat16)
    pewT = np.stack([pe_w[:, 0], pe_w[:, 1], pe_b]).astype(ml_dtypes.bfloat16)

    idx = np.arange(DH)
    in_maps = []
    for c in range(8):
        b, h = c // 4, c % 4
        rows = h + HEADS * idx
        qdiags = np.zeros((DIM, 9, DIM), np.float32)
        taps = qkv_dw.reshape(DIM, 9)
        for t in range(9):
            qdiags[np.arange(DIM), t, np.arange(DIM)] = taps[:, t]
        odiags = np.zeros((DH, 9, DH), np.float32)
        otaps = out_dw[rows].reshape(DH, 9)
        for t in range(9):
            odiags[idx, t, idx] = otaps[:, t]
        m = {
            "x": np.ascontiguousarray(x[b].reshape(DIM, N)),
            "meshb": meshb,
            "pewT": pewT,
            "qdiags": np.ascontiguousarray(qdiags.reshape(DIM, 9 * DIM)),
            "pwq": np.ascontiguousarray(qkv_pw[rows, :].T),
            "pwk": np.ascontiguousarray(qkv_pw[DIM * 2 + rows, :].T),
            "pwv": np.ascontiguousarray(qkv_pw[DIM * 4 + rows, :].T),
            "lnqw": np.ascontiguousarray(nq_w[h][None, :]),
            "lnqb": np.ascontiguousarray(nq_b[h][:, None]),
            "lnkw": np.ascontiguousarray(nk_w[h][None, :]),
            "lnkb": np.ascontiguousarray(nk_b[h][:, None]),
            "lnow": np.ascontiguousarray(no_w[h][None, :]),
            "lnob": np.ascontiguousarray(no_b[h][:, None]),
            "odiags": np.ascontiguousarray(odiags.reshape(DH, 9 * DH)).astype(
                ml_dtypes.bfloat16
            ),
            "opw": np.ascontiguousarray(out_pw[:, rows].T).astype(ml_dtypes.bfloat16),
            "ln2w": np.ascontiguousarray(ln_w[None, :]),
            "ln2b": np.ascontiguousarray(ln_b[None, :]),
            "o64h": np.full((DH, 1), 1.0 / DH, np.float32),
            "zpad": np.zeros((DIM, G), np.float32),
            "onesr": np.ones((1, DH), np.float32),
        }
        in_maps.append(m)
    return in_maps


def run_on_device(inputs, **kw):
    nc = _get_nc()
    in_maps = _make_in_maps(inputs)
    res = run_bass_kernel_spmd(nc, in_maps, core_ids=list(range(8)), **kw)
    out = np.zeros((B, DIM, N), np.float32)
    for c in range(8):
        b, h = c // 4, c % 4
        out[b][:, h * (N // 4) : (h + 1) * (N // 4)] = res.results[c]["out"].T
    return out.reshape(B, DIM, Hs, Ws), res


def kernel(**inputs):
    out, _ = run_on_device(inputs)
    return out


# revision 25
# speedup vs baseline: 1.3346x; 1.0009x over previous
"""Trainium2 Bass kernel for nn_Attention_19404662243470.

Sharding: 8 cores = (batch 2) x (heads 4). Each core computes the full
attention pipeline for its (b, h) pair in transposed layout [d, n]; the
final pointwise conv partials are ReduceScattered within each batch's
4-core group, and LayerNorm2d runs on each core's position shard.

Layout notes:
 - q/k/v come out of the pointwise conv directly as [d, n] ("T layout"),
   which is exactly the operand layout the S^T = K Q^T matmul needs.
 - softmax runs without max-subtraction (logits are bounded ~|5|); the
   denominator falls out of the AV matmul via an appended ones-row in V.
 - per-head LN over d (the partition dim) uses ones-matmuls for the
   stats and K=1 broadcast matmuls to spread per-column scalars.
 - S^T pairs are row-packed onto the two halves of the PE array
   (contraction is only 64 deep), doubling S^T throughput.
 - long PE idle gaps are avoided (HAM throttles the PE clock to 1.2 GHz
   after ~3.4us of idle and has been seen never to recover): LN scalar
   chains are overlapped with independent matmul work, and the out-LN
   is folded into the per-chunk attention loop.
"""

import numpy as np

import concourse.bass as bass
import concourse.tile as tile
from concourse import bacc, mybir
from concourse.bass_utils import run_bass_kernel_spmd

dt = mybir.dt
AF = mybir.ActivationFunctionType
OP = mybir.AluOpType

B, DIM, Hs, Ws = 2, 128, 64, 64
HEADS, DH = 4, 64
N = Hs * Ws  # 4096
EPS = 1e-6
IC = 512  # i-chunk width
NIC = N // IC  # 8
JB = 128  # j-block
NJB = N // JB  # 32
NCH = N // 128  # 32
G = Hs + 2  # 66 padded grid


def _build():
    nc = bacc.Bacc()

    def par(name, shape, dtyp=dt.float32):
        return nc.declare_dram_parameter(name, list(shape), dtyp, isOutput=False)

    x = par("x", [DIM, N])
    meshb = par("meshb", [3, N], dt.bfloat16)
    pewT = par("pewT", [3, DIM], dt.bfloat16)
    qdiags = par("qdiags", [DIM, 9 * DIM], dt.float32r)
    pwq = par("pwq", [DIM, DH], dt.float32r)
    pwk = par("pwk", [DIM, DH], dt.float32r)
    pwv = par("pwv", [DIM, DH], dt.float32r)
    lnqw = par("lnqw", [1, DH], dt.float32r)
    lnqb = par("lnqb", [DH, 1])
    lnkw = par("lnkw", [1, DH], dt.float32r)
    lnkb = par("lnkb", [DH, 1])
    lnow = par("lnow", [1, DH], dt.float32r)
    lnob = par("lnob", [DH, 1])
    odiags = par("odiags", [DH, 9 * DH], dt.bfloat16)
    opw = par("opw", [DH, DIM], dt.bfloat16)
    ln2w = par("ln2w", [1, DIM])
    ln2b = par("ln2b", [1, DIM])
    o64hd = par("o64h", [DH, 1], dt.float32r)
    zpad = par("zpad", [DIM, G], dt.float32r)
    onesrd = par("onesr", [1, DH], dt.float32r)
    out_ext = nc.declare_dram_parameter("out", [N // 4, DIM], dt.float32, isOutput=True)

    rs_in = nc.dram_tensor("rs_in", [N, DIM], dt.float32)
    rs_out = nc.dram_tensor("rs_out", [N // 4, DIM], dt.float32)

    with (
        nc.allow_low_precision(reason="float32r/bf16 compute by design"),
        tile.TileContext(nc) as tc,
        tc.tile_pool(name="main", bufs=1) as main,
        tc.tile_pool(name="tmp2", bufs=2) as tmp2,
    ):
        # ---- persistent SBUF tiles ----
        QL = main.tile([128, N], dt.float32r)  # LN'd q, duplicated on both halves
        KL = main.tile([128, N], dt.float32r)
        VT = main.tile([DH, N], dt.bfloat16)  # v^T for the skip connection
        V = main.tile([128, NCH, DH + 1], dt.bfloat16)
        SC = main.tile([1, 2 * N], dt.float32)  # mu | E2 (E2 becomes var/rs)
        SCB = main.tile([1, 2 * N], dt.float32r)  # rs | mu*rs (matmul-ready)
        Og = main.tile([DH, G, G], dt.bfloat16)  # padded out-LN grid
        o64h = main.tile([DH, 1], dt.float32r)
        nc.sync.dma_start(out=o64h, in_=o64hd[:, :])
        lnqb_t = main.tile([DH, 1], dt.float32)
        lnkb_t = main.tile([DH, 1], dt.float32)
        lnob_t = main.tile([DH, 1], dt.float32)
        nc.sync.dma_start(out=lnqb_t, in_=lnqb[:, :])
        nc.sync.dma_start(out=lnkb_t, in_=lnkb[:, :])
        nc.sync.dma_start(out=lnob_t, in_=lnob[:, :])
        lnqw_t = main.tile([1, DH], dt.float32r)
        lnkw_t = main.tile([1, DH], dt.float32r)
        lnow_t = main.tile([1, DH], dt.float32r)
        nc.sync.dma_start(out=lnqw_t, in_=lnqw[:, :])
        nc.sync.dma_start(out=lnkw_t, in_=lnkw[:, :])
        nc.sync.dma_start(out=lnow_t, in_=lnow[:, :])
        onesr = main.tile([1, DH], dt.float32r)
        nc.sync.dma_start(out=onesr, in_=onesrd[:, :])
        epsP = main.tile([128, 1], dt.float32)
        nc.vector.memset(epsP, EPS)
        nc.vector.memset(V, 1.0)
        nc.vector.memset(Og, 0.0)

        def stats_mms(psp, src_ap, c):
            """mu and E[x^2] rows for a [64, IC] chunk into SC columns c."""
            sq = tmp2.tile([DH, IC], dt.float32r, tag="sq")
            nc.vector.tensor_mul(out=sq, in0=src_ap, in1=src_ap)
            smu = psp.tile([1, IC], dt.float32, tag="smu", bufs=1)
            se2 = psp.tile([1, IC], dt.float32, tag="se2", bufs=1)
            nc.tensor.matmul(smu, o64h, src_ap, start=True, stop=True)
            nc.tensor.matmul(se2, o64h, sq, start=True, stop=True)
            nc.vector.tensor_copy(out=SC[:, c * IC : (c + 1) * IC], in_=smu)
            nc.vector.tensor_copy(out=SC[:, N + c * IC : N + (c + 1) * IC], in_=se2)

        def ln_chain(lo, hi):
            """SC mu/E2 -> SCB rs / mu*rs over columns [lo, hi)."""
            mu = SC[:, lo:hi]
            e2 = SC[:, N + lo : N + hi]
            mrs = SCB[:, N + lo : N + hi]
            rs = SCB[:, lo:hi]
            nc.vector.scalar_tensor_tensor(
                out=mrs, in0=mu, scalar=-1.0, in1=mu, op0=OP.mult, op1=OP.mult
            )
            nc.vector.tensor_add(out=e2, in0=e2, in1=mrs)
            # 1/sqrt(v+eps) = exp(-0.5*ln(v+eps)): keeps the Exp table set
            # resident (a Sqrt would force a table reload every chunk)
            nc.scalar.activation(out=e2, in_=e2, func=AF.Ln, bias=epsP[0:1, :])
            nc.scalar.activation(out=rs, in_=e2, func=AF.Exp, scale=-0.5)
            nc.vector.tensor_mul(out=mrs, in0=mu, in1=rs)

        def ln_bcast_apply(psp, w_row, b_t, src_ap, dst, c, dst_is_grid=False, bc_bufs=2):
            """dst chunk c = (src*rs - mu*rs)*w + b via two K=1 bcasts."""
            bcA = psp.tile([DH, IC], dt.float32, tag="bc", bufs=bc_bufs)
            nc.tensor.matmul(
                bcA, w_row, SCB[:, c * IC : (c + 1) * IC], start=True, stop=True
            )
            bcB = psp.tile([DH, IC], dt.float32, tag="bc", bufs=bc_bufs)
            nc.tensor.matmul(
                bcB, w_row, SCB[:, N + c * IC : N + (c + 1) * IC], start=True, stop=True
            )
            T = tmp2.tile([DH, IC], dt.float32, tag="T")
            nc.vector.tensor_mul(out=T, in0=src_ap, in1=bcA)
            if dst_is_grid:
                r0 = c * 8
                nc.vector.scalar_tensor_tensor(
                    out=dst[:, 1 + r0 : 9 + r0, 1 : 1 + Ws],
                    in0=T.rearrange("p (a b) -> p a b", b=Ws),
                    scalar=b_t,
                    in1=bcB.rearrange("p (a b) -> p a b", b=Ws),
                    op0=OP.add,
                    op1=OP.subtract,
                )
            else:
                nc.vector.scalar_tensor_tensor(
                    out=dst[0:DH, c * IC : (c + 1) * IC],
                    in0=T,
                    scalar=b_t,
                    in1=bcB,
                    op0=OP.add,
                    op1=OP.subtract,
                )

        # ============ Stage A1: pos + depthwise ============
        with tc.tile_pool(name="stageA", bufs=1) as pA:
            psA1cm = tc.tile_pool(name="psA1", bufs=2, space="PSUM")
            psA1 = psA1cm.__enter__()
            Xg = pA.tile([DIM, G, G], dt.float32r)
            pA0 = tc.tile_pool(name="pA0", bufs=1)
            pA0p = pA0.__enter__()
            msh = pA0p.tile([3, N], dt.bfloat16)
            nc.scalar.dma_start(out=msh, in_=meshb[:, :])
            pwt = pA0p.tile([3, DIM], dt.bfloat16)
            nc.scalar.dma_start(out=pwt, in_=pewT[:, :])
            nc.sync.dma_start(out=Xg[:, 0:1, :], in_=zpad[:, :].unsqueeze(1))
            nc.sync.dma_start(out=Xg[:, G - 1 : G, :], in_=zpad[:, :].unsqueeze(1))
            nc.scalar.dma_start(
                out=Xg[:, 1 : G - 1, 0:1], in_=zpad[:, 0 : G - 2].unsqueeze(2)
            )
            nc.scalar.dma_start(
                out=Xg[:, 1 : G - 1, G - 1 : G], in_=zpad[:, 0 : G - 2].unsqueeze(2)
            )
            for q in range(4):
                eng = nc.sync if q % 2 == 0 else nc.scalar
                eng.dma_start(
                    out=Xg[:, 1 + 16 * q : 1 + 16 * (q + 1), 1 : 1 + Ws],
                    in_=x[:, 1024 * q : 1024 * (q + 1)]
                    .bitcast(dt.float32r)
                    .rearrange("p (a b) -> p a b", b=Ws),
                )
            pwq_t = pA.tile([DIM, DH], dt.float32r)
            pwk_t = pA.tile([DIM, DH], dt.float32r)
            pwv_t = pA.tile([DIM, DH], dt.float32r)
            nc.sync.dma_start(out=pwq_t, in_=pwq[:, :])
            nc.sync.dma_start(out=pwk_t, in_=pwk[:, :])
            nc.sync.dma_start(out=pwv_t, in_=pwv[:, :])

            # pos encoding into the guard interior
            for c in range(NIC):
                pos_ps = psA1.tile([DIM, IC], dt.float32, tag="pos")
                nc.tensor.matmul(
                    pos_ps, pwt, msh[:, c * IC : (c + 1) * IC], start=True, stop=True
                )
                r0 = c * 8
                view = Xg[:, 1 + r0 : 9 + r0, 1 : 1 + Ws]
                nc.vector.tensor_add(
                    out=view, in0=view, in1=pos_ps.rearrange("p (a b) -> p a b", b=Ws)
                )

            pA0.__exit__(None, None, None)
            # depthwise 3x3 via 9 accumulated diag matmuls
            qdg = pA.tile([DIM, 9, DIM], dt.float32r)
            nc.scalar.dma_start(
                out=qdg, in_=qdiags[:, :].rearrange("p (t c) -> p t c", t=9)
            )
            Yr = pA.tile([DIM, N], dt.float32r)
            for c in range(NIC):
                dwp = psA1.tile([DIM, IC], dt.float32, tag="dw")
                r0 = c * 8
                t = 0
                for di in range(3):
                    for dj in range(3):
                        nc.tensor.matmul(
                            dwp,
                            qdg[:, t, :],
                            Xg[:, r0 + di : r0 + di + 8, dj : dj + Ws],
                            start=(t == 0),
                            stop=(t == 8),
                        )
                        t += 1
                nc.vector.tensor_copy(out=Yr[:, c * IC : (c + 1) * IC], in_=dwp)

            psA1cm.__exit__(None, None, None)
            # ============ Stage A2: pointwise + q/k LN + v ============
            with tc.tile_pool(name="psA2", bufs=1, space="PSUM") as psA2:
                QRW = pA.tile([DH, N], dt.float32r)
                KRAW = pA.tile([DH, N], dt.float32r)
                for c in range(NIC):
                    qp = psA2.tile([DH, IC], dt.float32, tag="qp", bufs=2)
                    nc.tensor.matmul(
                        qp, pwq_t, Yr[:, c * IC : (c + 1) * IC], start=True, stop=True
                    )
                    nc.vector.tensor_copy(out=QRW[:, c * IC : (c + 1) * IC], in_=qp)
                for c in range(NIC):
                    stats_mms(psA2, QRW[:, c * IC : (c + 1) * IC], c)
                # q chain on DVE/ACT runs while the PE does k's pointwise
                ln_chain(0, N)
                for c in range(NIC):
                    qp = psA2.tile([DH, IC], dt.float32, tag="qp", bufs=2)
                    nc.tensor.matmul(
                        qp, pwk_t, Yr[:, c * IC : (c + 1) * IC], start=True, stop=True
                    )
                    nc.vector.tensor_copy(out=KRAW[:, c * IC : (c + 1) * IC], in_=qp)

                # q broadcast + apply -> QL lower half, then mirror upward
                for c in range(NIC):
                    ln_bcast_apply(psA2, lnqw_t, lnqb_t, QRW[:, c * IC : (c + 1) * IC], QL, c)
                nc.vector.tensor_copy(out=QL[DH:128, :], in_=QL[0:DH, :])

                for c in range(NIC):
                    stats_mms(psA2, KRAW[:, c * IC : (c + 1) * IC], c)
                # k chain runs while the PE builds v (N-layout) and v^T
                ln_chain(0, N)
                for ch in range(NCH):
                    vp = psA2.tile([128, DH], dt.float32, tag="vp", bufs=2)
                    nc.tensor.matmul(
                        vp,
                        Yr[:, ch * 128 : (ch + 1) * 128],
                        pwv_t,
                        start=True,
                        stop=True,
                    )
                    nc.vector.tensor_copy(out=V[:, ch, 0:DH], in_=vp)
                for c in range(NIC):
                    qp = psA2.tile([DH, IC], dt.float32, tag="qp", bufs=2)
                    nc.tensor.matmul(
                        qp, pwv_t, Yr[:, c * IC : (c + 1) * IC], start=True, stop=True
                    )
                    nc.vector.tensor_copy(out=VT[:, c * IC : (c + 1) * IC], in_=qp)

                for c in range(NIC):
                    ln_bcast_apply(psA2, lnkw_t, lnkb_t, KRAW[:, c * IC : (c + 1) * IC], KL, c)
                nc.vector.tensor_copy(out=KL[DH:128, :], in_=KL[0:DH, :])

        # ============ Stage B: attention with inline out-LN ============
        with tc.tile_pool(name="psB", bufs=1, space="PSUM") as psB, tc.tile_pool(
            name="sbB", bufs=3
        ) as sbB:
            NG = NJB // 2
            pending_tail = []

            def attention_block(c):
                avp = psB.tile([DH + 1, IC], dt.float32, tag="avp", bufs=1)
                stgs = {}
                Es = {}

                def issue_st(g):
                    stg = psB.tile([128, 2 * IC], dt.float32, tag="stg", bufs=2)
                    j0 = 2 * g * JB
                    nc.tensor.matmul(
                        stg[:, 0:IC],
                        KL[0:DH, j0 : j0 + JB],
                        QL[0:DH, c * IC : (c + 1) * IC],
                        start=True,
                        stop=True,
                    )
                    nc.tensor.matmul(
                        stg[:, IC : 2 * IC],
                        KL[DH:128, j0 + JB : j0 + 2 * JB],
                        QL[DH:128, c * IC : (c + 1) * IC],
                        start=True,
                        stop=True,
                    )
                    stgs[g] = stg

                def issue_exp(g):
                    E = sbB.tile([128, 2 * IC], dt.bfloat16, tag="E")
                    nc.scalar.activation(
                        out=E, in_=stgs.pop(g), func=AF.Exp, scale=float(DH**-0.5)
                    )
                    Es[g] = E

                def issue_av(g):
                    E = Es.pop(g)
                    for t in range(2):
                        jb = 2 * g + t
                        nc.tensor.matmul(
                            avp,
                            V[:, jb, :],
                            E[:, t * IC : (t + 1) * IC],
                            start=(jb == 0),
                            stop=(jb == NJB - 1),
                            skip_group_check=True,
                        )

                issue_st(0)
                issue_exp(0)
                for g in range(1, NG):
                    issue_st(g)
                    issue_exp(g)
                    issue_av(g - 1)
                issue_av(NG - 1)

                # free avp quickly: park the numerator and denominator
                REC = sbB.tile([1, IC], dt.float32, tag="REC", bufs=2)
                nc.vector.tensor_copy(out=REC, in_=avp[DH : DH + 1, :])
                Tn = sbB.tile([DH, IC], dt.float32, tag="Tn", bufs=2)
                nc.vector.tensor_copy(out=Tn, in_=avp[0:DH, :])
                return REC, Tn

            def tail_block(c, REC, Tn):
                # DVE/ACT chain + the few tail matmuls for chunk c; issued
                # after the NEXT chunk's attention matmuls so the PE stream
                # never waits on the chain.
                nc.vector.reciprocal_approx_fast(out=REC, in_=REC)
                RECB = sbB.tile([1, IC], dt.float32r, tag="RECB", bufs=2)
                nc.vector.tensor_copy(out=RECB, in_=REC)
                bcR = psB.tile([DH, IC], dt.float32, tag="bc", bufs=1)
                nc.tensor.matmul(bcR, onesr, RECB, start=True, stop=True)
                OSc = sbB.tile([DH, IC], dt.float32r, tag="OS", bufs=2)
                nc.vector.tensor_mul(out=OSc, in0=Tn, in1=bcR)
                nc.vector.tensor_add(
                    out=OSc, in0=OSc, in1=VT[:, c * IC : (c + 1) * IC]
                )
                stats_mms(psB, OSc[:, :], c)
                ln_chain(c * IC, (c + 1) * IC)
                ln_bcast_apply(
                    psB, lnow_t, lnob_t, OSc[:, :], Og, c, dst_is_grid=True, bc_bufs=1
                )

            for c in range(NIC):
                rec_tn = attention_block(c)
                if pending_tail:
                    tail_block(*pending_tail.pop())
                pending_tail.append((c, *rec_tn))
            tail_block(*pending_tail.pop())

        # ============ Stage C: out depthwise + pointwise partial ============
        with (
            tc.tile_pool(name="stageC", bufs=1) as pC,
            tc.tile_pool(name="psC", bufs=2, space="PSUM") as psC,
        ):
            odg = pC.tile([DH, 9, DH], dt.bfloat16)
            nc.sync.dma_start(
                out=odg, in_=odiags[:, :].rearrange("p (t c) -> p t c", t=9)
            )
            opw_t = pC.tile([DH, DIM], dt.bfloat16)
            nc.sync.dma_start(out=opw_t, in_=opw[:, :])
            DWO = pC.tile([DH, N], dt.bfloat16)
            for c in range(NIC):
                dwp = psC.tile([DH, IC], dt.float32, tag="dw")
                r0 = c * 8
                t = 0
                for di in range(3):
                    for dj in range(3):
                        nc.tensor.matmul(
                            dwp,
                            odg[:, t, :],
                            Og[:, r0 + di : r0 + di + 8, dj : dj + Ws],
                            start=(t == 0),
                            stop=(t == 8),
                        )
                        t += 1
                nc.vector.tensor_copy(out=DWO[:, c * IC : (c + 1) * IC], in_=dwp)
            for ch in range(NCH):
                pp = psC.tile([128, DIM], dt.float32, tag="pp")
                nc.tensor.matmul(
                    pp, DWO[:, ch * 128 : (ch + 1) * 128], opw_t, start=True, stop=True
                )
                PP = tmp2.tile([128, DIM], dt.float32, tag="PP")
                nc.vector.tensor_copy(out=PP, in_=pp)
                nc.sync.dma_start(out=rs_in[ch * 128 : (ch + 1) * 128, :], in_=PP)

        # ============ Stage D: ReduceScatter + LayerNorm2d ============
        nc.gpsimd.collective_compute(
            "ReduceScatter",
            OP.add,
            replica_groups=[[0, 1, 2, 3], [4, 5, 6, 7]],
            ins=[rs_in[:, :]],
            outs=[rs_out[:, :]],
        )
        with tc.tile_pool(name="stageD", bufs=2) as pD:
            w_b = pD.tile([128, DIM], dt.float32, bufs=1)
            b_b = pD.tile([128, DIM], dt.float32, bufs=1)
            nc.sync.dma_start(out=w_b, in_=ln2w[:, :].to_broadcast([128, DIM]))
            nc.sync.dma_start(out=b_b, in_=ln2b[:, :].to_broadcast([128, DIM]))
            for tkn in range(8):
                R = pD.tile([128, DIM], dt.float32, tag="R")
                nc.sync.dma_start(out=R, in_=rs_out[tkn * 128 : (tkn + 1) * 128, :])
                st = pD.tile([128, 6], dt.float32, tag="st")
                nc.vector.bn_stats(out=st, in_=R)
                mv = pD.tile([128, 2], dt.float32, tag="mv")
                nc.vector.bn_aggr(out=mv, in_=st)
                sd = pD.tile([128, 1], dt.float32, tag="sd")
                nc.scalar.activation(out=sd, in_=mv[:, 1:2], func=AF.Ln, bias=epsP)
                nc.scalar.activation(out=sd, in_=sd, func=AF.Exp, scale=-0.5)
                nc.vector.tensor_scalar(
                    out=R,
                    in0=R,
                    scalar1=mv[:, 0:1],
                    scalar2=sd,
                    op0=OP.subtract,
                    op1=OP.mult,
                )
                R2 = pD.tile([128, DIM], dt.float32, tag="R2")
                nc.vector.tensor_mul(out=R2, in0=R, in1=w_b)
                nc.vector.tensor_add(out=R2, in0=R2, in1=b_b)
                nc.sync.dma_start(out=out_ext[tkn * 128 : (tkn + 1) * 128, :], in_=R2)

    return nc


_cached = {}


def _get_nc():
    if "nc" not in _cached:
        nc = _build()
        nc.finalize()
        _cached["nc"] = nc
    return _cached["nc"]


def _make_in_maps(inputs):
    import ml_dtypes

    x = np.asarray(inputs["x"], np.float32)
    pe_w = np.asarray(inputs["pe_w"], np.float32)
    pe_b = np.asarray(inputs["pe_b"], np.float32)
    qkv_dw = np.asarray(inputs["qkv_dw"], np.float32)
    qkv_pw = np.asarray(inputs["qkv_pw"], np.float32)
    out_dw = np.asarray(inputs["out_dw"], np.float32)
    out_pw = np.asarray(inputs["out_pw"], np.float32)
    nq_w, nq_b = np.asarray(inputs["nq_w"], np.float32), np.asarray(
        inputs["nq_b"], np.float32
    )
    nk_w, nk_b = np.asarray(inputs["nk_w"], np.float32), np.asarray(
        inputs["nk_b"], np.float32
    )
    no_w, no_b = np.asarray(inputs["no_w"], np.float32), np.asarray(
        inputs["no_b"], np.float32
    )
    ln_w, ln_b = np.asarray(inputs["ln_w"], np.float32), np.asarray(
        inputs["ln_b"], np.float32
    )

    gx = np.linspace(0.0, 1.0, Hs, dtype=np.float32)
    gy = np.linspace(0.0, 1.0, Ws, dtype=np.float32)
    meshb = np.stack(
        [np.repeat(gx, Ws), np.tile(gy, Hs), np.ones(N, np.float32)]
    ).astype(ml_dtypes.bfloat16)
    pewT = np.stack([pe_w[:, 0], pe_w[:, 1], pe_b]).astype(ml_dtypes.bfloat16)

    idx = np.arange(DH)
    in_maps = []
    for c in range(8):
        b, h = c // 4, c % 4
        rows = h + HEADS * idx
        qdiags = np.zeros((DIM, 9, DIM), np.float32)
        taps = qkv_dw.reshape(DIM, 9)
        for t in range(9):
            qdiags[np.arange(DIM), t, np.arange(DIM)] = taps[:, t]
        odiags = np.zeros((DH, 9, DH), np.float32)
        otaps = out_dw[rows].reshape(DH, 9)
        for t in range(9):
            odiags[idx, t, idx] = otaps[:, t]
        m = {
            "x": np.ascontiguousarray(x[b].reshape(DIM, N)),
            "meshb": meshb,
            "pewT": pewT,
            "qdiags": np.ascontiguousarray(qdiags.reshape(DIM, 9 * DIM)),
            "pwq": np.ascontiguousarray(qkv_pw[rows, :].T),
            "pwk": np.ascontiguousarray(qkv_pw[DIM * 2 + rows, :].T),
            "pwv": np.ascontiguousarray(qkv_pw[DIM * 4 + rows, :].T),
            "lnqw": np.ascontiguousarray(nq_w[h][None, :]),
            "lnqb": np.ascontiguousarray(nq_b[h][:, None]),
            "lnkw": np.ascontiguousarray(nk_w[h][None, :]),
            "lnkb": np.ascontiguousarray(nk_b[h][:, None]),
            "lnow": np.ascontiguousarray(no_w[h][None, :]),
            "lnob": np.ascontiguousarray(no_b[h][:, None]),
            "odiags": np.ascontiguousarray(odiags.reshape(DH, 9 * DH)).astype(
                ml_dtypes.bfloat16
            ),
            "opw": np.ascontiguousarray(out_pw[:, rows].T).astype(ml_dtypes.bfloat16),
            "ln2w": np.ascontiguousarray(ln_w[None, :]),
            "ln2b": np.ascontiguousarray(ln_b[None, :]),
            "o64h": np.full((DH, 1), 1.0 / DH, np.float32),
            "zpad": np.zeros((DIM, G), np.float32),
            "onesr": np.ones((1, DH), np.float32),
        }
        in_maps.append(m)
    return in_maps


def run_on_device(inputs, **kw):
    nc = _get_nc()
    in_maps = _make_in_maps(inputs)
    res = run_bass_kernel_spmd(nc, in_maps, core_ids=list(range(8)), **kw)
    out = np.zeros((B, DIM, N), np.float32)
    for c in range(8):
        b, h = c // 4, c % 4
        out[b][:, h * (N // 4) : (h + 1) * (N // 4)] = res.results[c]["out"].T
    return out.reshape(B, DIM, Hs, Ws), res


def kernel(**inputs):
    out, _ = run_on_device(inputs)
    return out


# revision 26
# speedup vs baseline: 1.4772x; 1.1068x over previous
"""Trainium2 Bass kernel for nn_Attention_19404662243470.

Sharding: 8 cores = (batch 2) x (heads 4). Each core computes the full
attention pipeline for its (b, h) pair in transposed layout [d, n]; the
final pointwise conv partials are ReduceScattered within each batch's
4-core group, and LayerNorm2d runs on each core's position shard.

Layout notes:
 - q/k/v come out of the pointwise conv directly as [d, n] ("T layout"),
   which is exactly the operand layout the S^T = K Q^T matmul needs.
 - softmax runs without max-subtraction (logits are bounded ~|5|); the
   denominator falls out of the AV matmul via an appended ones-row in V.
 - per-head LN over d (the partition dim) uses ones-matmuls for the
   stats and K=1 broadcast matmuls to spread per-column scalars.
 - S^T pairs are row-packed onto the two halves of the PE array
   (contraction is only 64 deep), doubling S^T throughput.
 - long PE idle gaps are avoided (HAM throttles the PE clock to 1.2 GHz
   after ~3.4us of idle and has been seen never to recover): LN scalar
   chains are overlapped with independent matmul work, and the out-LN
   is folded into the per-chunk attention loop.
"""

import numpy as np

import concourse.bass as bass
import concourse.tile as tile
from concourse import bacc, mybir
from concourse.bass_utils import run_bass_kernel_spmd

dt = mybir.dt
AF = mybir.ActivationFunctionType
OP = mybir.AluOpType

B, DIM, Hs, Ws = 2, 128, 64, 64
HEADS, DH = 4, 64
N = Hs * Ws  # 4096
EPS = 1e-6
IC = 512  # i-chunk width
NIC = N // IC  # 8
JB = 128  # j-block
NJB = N // JB  # 32
NCH = N // 128  # 32
G = Hs + 2  # 66 padded grid


_TABLES_PATCHED = False


def _patch_act_tables():
    """Restrict Exp/Ln to the natural_log_exp_and_others set so the ACT
    table never reloads between the softmax Exp stream and the LN-chain
    Ln/Exp pairs (a reload costs ~1.3us and stalls the PE's exp feed)."""
    global _TABLES_PATCHED
    if _TABLES_PATCHED:
        return
    from concourse import bacc as _bacc_mod

    orig = _bacc_mod.get_activation_tables

    def patched(arch):
        tabs = dict(orig(arch))
        keep = {mybir.ActivationFunctionType.Exp, mybir.ActivationFunctionType.Ln}
        return {
            name: (fns if name == "natural_log_exp_and_others" else fns - keep)
            for name, fns in tabs.items()
        }

    _bacc_mod.get_activation_tables = patched
    _TABLES_PATCHED = True


def _build():
    _patch_act_tables()
    nc = bacc.Bacc()

    def par(name, shape, dtyp=dt.float32):
        return nc.declare_dram_parameter(name, list(shape), dtyp, isOutput=False)

    x = par("x", [DIM, N])
    meshb = par("meshb", [3, N], dt.bfloat16)
    pewT = par("pewT", [3, DIM], dt.bfloat16)
    qdiags = par("qdiags", [DIM, 9 * DIM], dt.float32r)
    pwq = par("pwq", [DIM, DH], dt.float32r)
    pwk = par("pwk", [DIM, DH], dt.float32r)
    pwv = par("pwv", [DIM, DH], dt.float32r)
    lnqw = par("lnqw", [1, DH], dt.float32r)
    lnqb = par("lnqb", [DH, 1])
    lnkw = par("lnkw", [1, DH], dt.float32r)
    lnkb = par("lnkb", [DH, 1])
    lnow = par("lnow", [1, DH], dt.float32r)
    lnob = par("lnob", [DH, 1])
    odiags = par("odiags", [DH, 9 * DH], dt.bfloat16)
    opw = par("opw", [DH, DIM], dt.bfloat16)
    ln2w = par("ln2w", [1, DIM])
    ln2b = par("ln2b", [1, DIM])
    o64hd = par("o64h", [DH, 1], dt.float32r)
    zpad = par("zpad", [DIM, G], dt.float32r)
    onesrd = par("onesr", [1, DH], dt.float32r)
    out_ext = nc.declare_dram_parameter("out", [N // 4, DIM], dt.float32, isOutput=True)

    rs_in = nc.dram_tensor("rs_in", [N, DIM], dt.float32)
    rs_out = nc.dram_tensor("rs_out", [N // 4, DIM], dt.float32)

    with (
        nc.allow_low_precision(reason="float32r/bf16 compute by design"),
        tile.TileContext(nc) as tc,
        tc.tile_pool(name="main", bufs=1) as main,
        tc.tile_pool(name="tmp2", bufs=2) as tmp2,
    ):
        # ---- persistent SBUF tiles ----
        QL = main.tile([128, N], dt.float32r)  # LN'd q, duplicated on both halves
        KL = main.tile([128, N], dt.float32r)
        VT = main.tile([DH, N], dt.bfloat16)  # v^T for the skip connection
        V = main.tile([128, NCH, DH + 1], dt.bfloat16)
        SC = main.tile([1, 2 * N], dt.float32)  # mu | E2 (E2 becomes var/rs)
        SCB = main.tile([1, 2 * N], dt.float32r)  # rs | mu*rs (matmul-ready)
        Og = main.tile([DH, G, G], dt.bfloat16)  # padded out-LN grid
        o64h = main.tile([DH, 1], dt.float32r)
        nc.sync.dma_start(out=o64h, in_=o64hd[:, :])
        lnqb_t = main.tile([DH, 1], dt.float32)
        lnkb_t = main.tile([DH, 1], dt.float32)
        lnob_t = main.tile([DH, 1], dt.float32)
        nc.sync.dma_start(out=lnqb_t, in_=lnqb[:, :])
        nc.sync.dma_start(out=lnkb_t, in_=lnkb[:, :])
        nc.sync.dma_start(out=lnob_t, in_=lnob[:, :])
        lnqw_t = main.tile([1, DH], dt.float32r)
        lnkw_t = main.tile([1, DH], dt.float32r)
        lnow_t = main.tile([1, DH], dt.float32r)
        nc.sync.dma_start(out=lnqw_t, in_=lnqw[:, :])
        nc.sync.dma_start(out=lnkw_t, in_=lnkw[:, :])
        nc.sync.dma_start(out=lnow_t, in_=lnow[:, :])
        onesr = main.tile([1, DH], dt.float32r)
        nc.sync.dma_start(out=onesr, in_=onesrd[:, :])
        epsP = main.tile([128, 1], dt.float32)
        nc.vector.memset(epsP, EPS)
        nc.vector.memset(V, 1.0)
        nc.vector.memset(Og, 0.0)

        def stats_mms(psp, src_ap, c):
            """mu and E[x^2] rows for a [64, IC] chunk into SC columns c."""
            sq = tmp2.tile([DH, IC], dt.float32r, tag="sq")
            nc.vector.tensor_mul(out=sq, in0=src_ap, in1=src_ap)
            smu = psp.tile([1, IC], dt.float32, tag="smu", bufs=1)
            se2 = psp.tile([1, IC], dt.float32, tag="se2", bufs=1)
            nc.tensor.matmul(smu, o64h, src_ap, start=True, stop=True)
            nc.tensor.matmul(se2, o64h, sq, start=True, stop=True)
            nc.vector.tensor_copy(out=SC[:, c * IC : (c + 1) * IC], in_=smu)
            nc.vector.tensor_copy(out=SC[:, N + c * IC : N + (c + 1) * IC], in_=se2)

        def ln_chain(lo, hi):
            """SC mu/E2 -> SCB rs / mu*rs over columns [lo, hi)."""
            mu = SC[:, lo:hi]
            e2 = SC[:, N + lo : N + hi]
            mrs = SCB[:, N + lo : N + hi]
            rs = SCB[:, lo:hi]
            nc.vector.scalar_tensor_tensor(
                out=mrs, in0=mu, scalar=-1.0, in1=mu, op0=OP.mult, op1=OP.mult
            )
            nc.vector.tensor_add(out=e2, in0=e2, in1=mrs)
            # 1/sqrt(v+eps) = exp(-0.5*ln(v+eps)): keeps the Exp table set
            # resident (a Sqrt would force a table reload every chunk)
            nc.scalar.activation(out=e2, in_=e2, func=AF.Ln, bias=epsP[0:1, :])
            nc.scalar.activation(out=rs, in_=e2, func=AF.Exp, scale=-0.5)
            nc.vector.tensor_mul(out=mrs, in0=mu, in1=rs)

        def ln_bcast_apply(psp, w_row, b_t, src_ap, dst, c, dst_is_grid=False, bc_bufs=2):
            """dst chunk c = (src*rs - mu*rs)*w + b via two K=1 bcasts."""
            bcA = psp.tile([DH, IC], dt.float32, tag="bc", bufs=bc_bufs)
            nc.tensor.matmul(
                bcA, w_row, SCB[:, c * IC : (c + 1) * IC], start=True, stop=True
            )
            bcB = psp.tile([DH, IC], dt.float32, tag="bc", bufs=bc_bufs)
            nc.tensor.matmul(
                bcB, w_row, SCB[:, N + c * IC : N + (c + 1) * IC], start=True, stop=True
            )
            T = tmp2.tile([DH, IC], dt.float32, tag="T")
            nc.vector.tensor_mul(out=T, in0=src_ap, in1=bcA)
            if dst_is_grid:
                r0 = c * 8
                nc.vector.scalar_tensor_tensor(
                    out=dst[:, 1 + r0 : 9 + r0, 1 : 1 + Ws],
                    in0=T.rearrange("p (a b) -> p a b", b=Ws),
                    scalar=b_t,
                    in1=bcB.rearrange("p (a b) -> p a b", b=Ws),
                    op0=OP.add,
                    op1=OP.subtract,
                )
            else:
                nc.vector.scalar_tensor_tensor(
                    out=dst[0:DH, c * IC : (c + 1) * IC],
                    in0=T,
                    scalar=b_t,
                    in1=bcB,
                    op0=OP.add,
                    op1=OP.subtract,
                )

        # ============ Stage A1: pos + depthwise ============
        with tc.tile_pool(name="stageA", bufs=1) as pA:
            psA1cm = tc.tile_pool(name="psA1", bufs=2, space="PSUM")
            psA1 = psA1cm.__enter__()
            Xg = pA.tile([DIM, G, G], dt.float32r)
            pA0 = tc.tile_pool(name="pA0", bufs=1)
            pA0p = pA0.__enter__()
            msh = pA0p.tile([3, N], dt.bfloat16)
            nc.scalar.dma_start(out=msh, in_=meshb[:, :])
            pwt = pA0p.tile([3, DIM], dt.bfloat16)
            nc.scalar.dma_start(out=pwt, in_=pewT[:, :])
            nc.sync.dma_start(out=Xg[:, 0:1, :], in_=zpad[:, :].unsqueeze(1))
            nc.sync.dma_start(out=Xg[:, G - 1 : G, :], in_=zpad[:, :].unsqueeze(1))
            nc.scalar.dma_start(
                out=Xg[:, 1 : G - 1, 0:1], in_=zpad[:, 0 : G - 2].unsqueeze(2)
            )
            nc.scalar.dma_start(
                out=Xg[:, 1 : G - 1, G - 1 : G], in_=zpad[:, 0 : G - 2].unsqueeze(2)
            )
            for q in range(4):
                eng = nc.sync if q % 2 == 0 else nc.scalar
                eng.dma_start(
                    out=Xg[:, 1 + 16 * q : 1 + 16 * (q + 1), 1 : 1 + Ws],
                    in_=x[:, 1024 * q : 1024 * (q + 1)]
                    .bitcast(dt.float32r)
                    .rearrange("p (a b) -> p a b", b=Ws),
                )
            pwq_t = pA.tile([DIM, DH], dt.float32r)
            pwk_t = pA.tile([DIM, DH], dt.float32r)
            pwv_t = pA.tile([DIM, DH], dt.float32r)
            nc.sync.dma_start(out=pwq_t, in_=pwq[:, :])
            nc.sync.dma_start(out=pwk_t, in_=pwk[:, :])
            nc.sync.dma_start(out=pwv_t, in_=pwv[:, :])

            # pos encoding into the guard interior
            for c in range(NIC):
                pos_ps = psA1.tile([DIM, IC], dt.float32, tag="pos")
                nc.tensor.matmul(
                    pos_ps, pwt, msh[:, c * IC : (c + 1) * IC], start=True, stop=True
                )
                r0 = c * 8
                view = Xg[:, 1 + r0 : 9 + r0, 1 : 1 + Ws]
                nc.vector.tensor_add(
                    out=view, in0=view, in1=pos_ps.rearrange("p (a b) -> p a b", b=Ws)
                )

            pA0.__exit__(None, None, None)
            # depthwise 3x3 via 9 accumulated diag matmuls
            qdg = pA.tile([DIM, 9, DIM], dt.float32r)
            nc.scalar.dma_start(
                out=qdg, in_=qdiags[:, :].rearrange("p (t c) -> p t c", t=9)
            )
            Yr = pA.tile([DIM, N], dt.float32r)
            for c in range(NIC):
                dwp = psA1.tile([DIM, IC], dt.float32, tag="dw")
                r0 = c * 8
                t = 0
                for di in range(3):
                    for dj in range(3):
                        nc.tensor.matmul(
                            dwp,
                            qdg[:, t, :],
                            Xg[:, r0 + di : r0 + di + 8, dj : dj + Ws],
                            start=(t == 0),
                            stop=(t == 8),
                        )
                        t += 1
                nc.vector.tensor_copy(out=Yr[:, c * IC : (c + 1) * IC], in_=dwp)

            psA1cm.__exit__(None, None, None)
            # ============ Stage A2: pointwise + q/k LN + v ============
            with tc.tile_pool(name="psA2", bufs=1, space="PSUM") as psA2:
                QRW = pA.tile([DH, N], dt.float32r)
                KRAW = pA.tile([DH, N], dt.float32r)
                for c in range(NIC):
                    qp = psA2.tile([DH, IC], dt.float32, tag="qp", bufs=2)
                    nc.tensor.matmul(
                        qp, pwq_t, Yr[:, c * IC : (c + 1) * IC], start=True, stop=True
                    )
                    nc.vector.tensor_copy(out=QRW[:, c * IC : (c + 1) * IC], in_=qp)
                for c in range(NIC):
                    stats_mms(psA2, QRW[:, c * IC : (c + 1) * IC], c)
                # q chain on DVE/ACT runs while the PE does k's pointwise
                ln_chain(0, N)
                for c in range(NIC):
                    qp = psA2.tile([DH, IC], dt.float32, tag="qp", bufs=2)
                    nc.tensor.matmul(
                        qp, pwk_t, Yr[:, c * IC : (c + 1) * IC], start=True, stop=True
                    )
                    nc.vector.tensor_copy(out=KRAW[:, c * IC : (c + 1) * IC], in_=qp)

                # q broadcast + apply -> QL lower half, then mirror upward
                for c in range(NIC):
                    ln_bcast_apply(psA2, lnqw_t, lnqb_t, QRW[:, c * IC : (c + 1) * IC], QL, c)
                nc.vector.tensor_copy(out=QL[DH:128, :], in_=QL[0:DH, :])

                for c in range(NIC):
                    stats_mms(psA2, KRAW[:, c * IC : (c + 1) * IC], c)
                # k chain runs while the PE builds v (N-layout) and v^T
                ln_chain(0, N)
                for ch in range(NCH):
                    vp = psA2.tile([128, DH], dt.float32, tag="vp", bufs=2)
                    nc.tensor.matmul(
                        vp,
                        Yr[:, ch * 128 : (ch + 1) * 128],
                        pwv_t,
                        start=True,
                        stop=True,
                    )
                    nc.vector.tensor_copy(out=V[:, ch, 0:DH], in_=vp)
                for c in range(NIC):
                    qp = psA2.tile([DH, IC], dt.float32, tag="qp", bufs=2)
                    nc.tensor.matmul(
                        qp, pwv_t, Yr[:, c * IC : (c + 1) * IC], start=True, stop=True
                    )
                    nc.vector.tensor_copy(out=VT[:, c * IC : (c + 1) * IC], in_=qp)

                for c in range(NIC):
                    ln_bcast_apply(psA2, lnkw_t, lnkb_t, KRAW[:, c * IC : (c + 1) * IC], KL, c)
                nc.vector.tensor_copy(out=KL[DH:128, :], in_=KL[0:DH, :])

        # ============ Stage B: attention with inline out-LN ============
        with tc.tile_pool(name="psB", bufs=1, space="PSUM") as psB, tc.tile_pool(
            name="sbB", bufs=3
        ) as sbB:
            NG = NJB // 2
            pending_tail = []

            def attention_block(c):
                avp = psB.tile([DH + 1, IC], dt.float32, tag="avp", bufs=1)
                stgs = {}
                Es = {}

                def issue_st(g):
                    stg = psB.tile([128, 2 * IC], dt.float32, tag="stg", bufs=2)
                    j0 = 2 * g * JB
                    nc.tensor.matmul(
                        stg[:, 0:IC],
                        KL[0:DH, j0 : j0 + JB],
                        QL[0:DH, c * IC : (c + 1) * IC],
                        start=True,
                        stop=True,
                    )
                    nc.tensor.matmul(
                        stg[:, IC : 2 * IC],
                        KL[DH:128, j0 + JB : j0 + 2 * JB],
                        QL[DH:128, c * IC : (c + 1) * IC],
                        start=True,
                        stop=True,
                    )
                    stgs[g] = stg

                def issue_exp(g):
                    E = sbB.tile([128, 2 * IC], dt.bfloat16, tag="E")
                    nc.scalar.activation(
                        out=E, in_=stgs.pop(g), func=AF.Exp, scale=float(DH**-0.5)
                    )
                    Es[g] = E

                def issue_av(g):
                    E = Es.pop(g)
                    for t in range(2):
                        jb = 2 * g + t
                        nc.tensor.matmul(
                            avp,
                            V[:, jb, :],
                            E[:, t * IC : (t + 1) * IC],
                            start=(jb == 0),
                            stop=(jb == NJB - 1),
                            skip_group_check=True,
                        )

                issue_st(0)
                issue_exp(0)
                for g in range(1, NG):
                    issue_st(g)
                    issue_exp(g)
                    issue_av(g - 1)
                issue_av(NG - 1)

                # free avp quickly: park the numerator and denominator
                REC = sbB.tile([1, IC], dt.float32, tag="REC", bufs=2)
                nc.vector.tensor_copy(out=REC, in_=avp[DH : DH + 1, :])
                Tn = sbB.tile([DH, IC], dt.float32, tag="Tn", bufs=2)
                nc.vector.tensor_copy(out=Tn, in_=avp[0:DH, :])
                return REC, Tn

            def tail_block(c, REC, Tn):
                # DVE/ACT chain + the few tail matmuls for chunk c; issued
                # after the NEXT chunk's attention matmuls so the PE stream
                # never waits on the chain.
                nc.vector.reciprocal_approx_fast(out=REC, in_=REC)
                RECB = sbB.tile([1, IC], dt.float32r, tag="RECB", bufs=2)
                nc.vector.tensor_copy(out=RECB, in_=REC)
                bcR = psB.tile([DH, IC], dt.float32, tag="bc", bufs=1)
                nc.tensor.matmul(bcR, onesr, RECB, start=True, stop=True)
                OSc = sbB.tile([DH, IC], dt.float32r, tag="OS", bufs=2)
                nc.vector.tensor_mul(out=OSc, in0=Tn, in1=bcR)
                nc.vector.tensor_add(
                    out=OSc, in0=OSc, in1=VT[:, c * IC : (c + 1) * IC]
                )
                stats_mms(psB, OSc[:, :], c)
                ln_chain(c * IC, (c + 1) * IC)
                ln_bcast_apply(
                    psB, lnow_t, lnob_t, OSc[:, :], Og, c, dst_is_grid=True, bc_bufs=1
                )

            for c in range(NIC):
                rec_tn = attention_block(c)
                if pending_tail:
                    tail_block(*pending_tail.pop())
                pending_tail.append((c, *rec_tn))
            tail_block(*pending_tail.pop())

        # ============ Stage C: out depthwise + pointwise partial ============
        with (
            tc.tile_pool(name="stageC", bufs=1) as pC,
            tc.tile_pool(name="psC", bufs=2, space="PSUM") as psC,
        ):
            odg = pC.tile([DH, 9, DH], dt.bfloat16)
            nc.sync.dma_start(
                out=odg, in_=odiags[:, :].rearrange("p (t c) -> p t c", t=9)
            )
            opw_t = pC.tile([DH, DIM], dt.bfloat16)
            nc.sync.dma_start(out=opw_t, in_=opw[:, :])
            DWO = pC.tile([DH, N], dt.bfloat16)
            for c in range(NIC):
                dwp = psC.tile([DH, IC], dt.float32, tag="dw")
                r0 = c * 8
                t = 0
                for di in range(3):
                    for dj in range(3):
                        nc.tensor.matmul(
                            dwp,
                            odg[:, t, :],
                            Og[:, r0 + di : r0 + di + 8, dj : dj + Ws],
                            start=(t == 0),
                            stop=(t == 8),
                        )
                        t += 1
                nc.vector.tensor_copy(out=DWO[:, c * IC : (c + 1) * IC], in_=dwp)
            for ch in range(NCH):
                pp = psC.tile([128, DIM], dt.float32, tag="pp")
                nc.tensor.matmul(
                    pp, DWO[:, ch * 128 : (ch + 1) * 128], opw_t, start=True, stop=True
                )
                PP = tmp2.tile([128, DIM], dt.float32, tag="PP")
                nc.vector.tensor_copy(out=PP, in_=pp)
                nc.sync.dma_start(out=rs_in[ch * 128 : (ch + 1) * 128, :], in_=PP)

        # ============ Stage D: ReduceScatter + LayerNorm2d ============
        nc.gpsimd.collective_compute(
            "ReduceScatter",
            OP.add,
            replica_groups=[[0, 1, 2, 3], [4, 5, 6, 7]],
            ins=[rs_in[:, :]],
            outs=[rs_out[:, :]],
        )
        with tc.tile_pool(name="stageD", bufs=2) as pD:
            w_b = pD.tile([128, DIM], dt.float32, bufs=1)
            b_b = pD.tile([128, DIM], dt.float32, bufs=1)
            nc.sync.dma_start(out=w_b, in_=ln2w[:, :].to_broadcast([128, DIM]))
            nc.sync.dma_start(out=b_b, in_=ln2b[:, :].to_broadcast([128, DIM]))
            for tkn in range(8):
                R = pD.tile([128, DIM], dt.float32, tag="R")
                nc.sync.dma_start(out=R, in_=rs_out[tkn * 128 : (tkn + 1) * 128, :])
                st = pD.tile([128, 6], dt.float32, tag="st")
                nc.vector.bn_stats(out=st, in_=R)
                mv = pD.tile([128, 2], dt.float32, tag="mv")
                nc.vector.bn_aggr(out=mv, in_=st)
                sd = pD.tile([128, 1], dt.float32, tag="sd")
                nc.scalar.activation(out=sd, in_=mv[:, 1:2], func=AF.Ln, bias=epsP)
                nc.scalar.activation(out=sd, in_=sd, func=AF.Exp, scale=-0.5)
                nc.vector.tensor_scalar(
                    out=R,
                    in0=R,
                    scalar1=mv[:, 0:1],
                    scalar2=sd,
                    op0=OP.subtract,
                    op1=OP.mult,
                )
                R2 = pD.tile([128, DIM], dt.float32, tag="R2")
                nc.vector.tensor_mul(out=R2, in0=R, in1=w_b)
                nc.vector.tensor_add(out=R2, in0=R2, in1=b_b)
                nc.sync.dma_start(out=out_ext[tkn * 128 : (tkn + 1) * 128, :], in_=R2)

    return nc


_cached = {}


def _get_nc():
    if "nc" not in _cached:
        nc = _build()
        nc.finalize()
        _cached["nc"] = nc
    return _cached["nc"]


def _make_in_maps(inputs):
    import ml_dtypes

    x = np.asarray(inputs["x"], np.float32)
    pe_w = np.asarray(inputs["pe_w"], np.float32)
    pe_b = np.asarray(inputs["pe_b"], np.float32)
    qkv_dw = np.asarray(inputs["qkv_dw"], np.float32)
    qkv_pw = np.asarray(inputs["qkv_pw"], np.float32)
    out_dw = np.asarray(inputs["out_dw"], np.float32)
    out_pw = np.asarray(inputs["out_pw"], np.float32)
    nq_w, nq_b = np.asarray(inputs["nq_w"], np.float32), np.asarray(
        inputs["nq_b"], np.float32
    )
    nk_w, nk_b = np.asarray(inputs["nk_w"], np.float32), np.asarray(
        inputs["nk_b"], np.float32
    )
    no_w, no_b = np.asarray(inputs["no_w"], np.float32), np.asarray(
        inputs["no_b"], np.float32
    )
    ln_w, ln_b = np.asarray(inputs["ln_w"], np.float32), np.asarray(
        inputs["ln_b"], np.float32
    )

    gx = np.linspace(0.0, 1.0, Hs, dtype=np.float32)
    gy = np.linspace(0.0, 1.0, Ws, dtype=np.float32)
    meshb = np.stack(
        [np.repeat(gx, Ws), np.tile(gy, Hs), np.ones(N, np.float32)]
    ).astype(ml_dtypes.bfloat16)
    pewT = np.stack([pe_w[:, 0], pe_w[:, 1], pe_b]).astype(ml_dtypes.bfloat16)

    idx = np.arange(DH)
    in_maps = []
    for c in range(8):
        b, h = c // 4, c % 4
        rows = h + HEADS * idx
        qdiags = np.zeros((DIM, 9, DIM), np.float32)
        taps = qkv_dw.reshape(DIM, 9)
        for t in range(9):
            qdiags[np.arange(DIM), t, np.arange(DIM)] = taps[:, t]
        odiags = np.zeros((DH, 9, DH), np.float32)
        otaps = out_dw[rows].reshape(DH, 9)
        for t in range(9):
            odiags[idx, t, idx] = otaps[:, t]
        m = {
            "x": np.ascontiguousarray(x[b].reshape(DIM, N)),
            "meshb": meshb,
            "pewT": pewT,
            "qdiags": np.ascontiguousarray(qdiags.reshape(DIM, 9 * DIM)),
            "pwq": np.ascontiguousarray(qkv_pw[rows, :].T),
            "pwk": np.ascontiguousarray(qkv_pw[DIM * 2 + rows, :].T),
            "pwv": np.ascontiguousarray(qkv_pw[DIM * 4 + rows, :].T),
            "lnqw": np.ascontiguousarray(nq_w[h][None, :]),
            "lnqb": np.ascontiguousarray(nq_b[h][:, None]),
            "lnkw": np.ascontiguousarray(nk_w[h][None, :]),
            "lnkb": np.ascontiguousarray(nk_b[h][:, None]),
            "lnow": np.ascontiguousarray(no_w[h][None, :]),
            "lnob": np.ascontiguousarray(no_b[h][:, None]),
            "odiags": np.ascontiguousarray(odiags.reshape(DH, 9 * DH)).astype(
                ml_dtypes.bfloat16
            ),
            "opw": np.ascontiguousarray(out_pw[:, rows].T).astype(ml_dtypes.bfloat16),
            "ln2w": np.ascontiguousarray(ln_w[None, :]),
            "ln2b": np.ascontiguousarray(ln_b[None, :]),
            "o64h": np.full((DH, 1), 1.0 / DH, np.float32),
            "zpad": np.zeros((DIM, G), np.float32),
            "onesr": np.ones((1, DH), np.float32),
        }
        in_maps.append(m)
    return in_maps


def run_on_device(inputs, **kw):
    nc = _get_nc()
    in_maps = _make_in_maps(inputs)
    res = run_bass_kernel_spmd(nc, in_maps, core_ids=list(range(8)), **kw)
    out = np.zeros((B, DIM, N), np.float32)
    for c in range(8):
        b, h = c // 4, c % 4
        out[b][:, h * (N // 4) : (h + 1) * (N // 4)] = res.results[c]["out"].T
    return out.reshape(B, DIM, Hs, Ws), res


def kernel(**inputs):
    out, _ = run_on_device(inputs)
    return out


# revision 30
# speedup vs baseline: 1.5361x; 1.0399x over previous
"""Trainium2 Bass kernel for nn_Attention_19404662243470.

Sharding: 8 cores = (batch 2) x (heads 4). Each core computes the full
attention pipeline for its (b, h) pair in transposed layout [d, n]; the
final pointwise conv partials are ReduceScattered within each batch's
4-core group, and LayerNorm2d runs on each core's position shard.

Layout notes:
 - q/k/v come out of the pointwise conv directly as [d, n] ("T layout"),
   which is exactly the operand layout the S^T = K Q^T matmul needs.
 - softmax runs without max-subtraction (logits are bounded ~|5|); the
   denominator falls out of the AV matmul via an appended ones-row in V.
 - per-head LN over d (the partition dim) uses ones-matmuls for the
   stats and K=1 broadcast matmuls to spread per-column scalars.
 - S^T pairs are row-packed onto the two halves of the PE array
   (contraction is only 64 deep), doubling S^T throughput.
 - long PE idle gaps are avoided (HAM throttles the PE clock to 1.2 GHz
   after ~3.4us of idle and has been seen never to recover): LN scalar
   chains are overlapped with independent matmul work, and the out-LN
   is folded into the per-chunk attention loop.
"""

import numpy as np

import concourse.bass as bass
import concourse.tile as tile
from concourse import bacc, mybir
from concourse.bass_utils import run_bass_kernel_spmd

dt = mybir.dt
AF = mybir.ActivationFunctionType
OP = mybir.AluOpType

B, DIM, Hs, Ws = 2, 128, 64, 64
HEADS, DH = 4, 64
N = Hs * Ws  # 4096
EPS = 1e-6
IC = 512  # i-chunk width
NIC = N // IC  # 8
JB = 128  # j-block
NJB = N // JB  # 32
NCH = N // 128  # 32
G = Hs + 2  # 66 padded grid


_TABLES_PATCHED = False


def _patch_act_tables():
    """Restrict Exp/Ln to the natural_log_exp_and_others set so the ACT
    table never reloads between the softmax Exp stream and the LN-chain
    Ln/Exp pairs (a reload costs ~1.3us and stalls the PE's exp feed)."""
    global _TABLES_PATCHED
    if _TABLES_PATCHED:
        return
    from concourse import bacc as _bacc_mod

    orig = _bacc_mod.get_activation_tables

    def patched(arch):
        tabs = dict(orig(arch))
        keep = {mybir.ActivationFunctionType.Exp, mybir.ActivationFunctionType.Ln}
        return {
            name: (fns if name == "natural_log_exp_and_others" else fns - keep)
            for name, fns in tabs.items()
        }

    _bacc_mod.get_activation_tables = patched
    _TABLES_PATCHED = True


def _build():
    _patch_act_tables()
    nc = bacc.Bacc()

    def par(name, shape, dtyp=dt.float32):
        return nc.declare_dram_parameter(name, list(shape), dtyp, isOutput=False)

    x = par("x", [DIM, N])
    meshb = par("meshb", [3, N], dt.bfloat16)
    pewT = par("pewT", [3, DIM], dt.bfloat16)
    qdiags = par("qdiags", [DIM, 9 * DIM], dt.float32r)
    pwq = par("pwq", [DIM, DH], dt.float32r)
    pwk = par("pwk", [DIM, DH], dt.float32r)
    pwv = par("pwv", [DIM, DH], dt.float32r)
    lnqw = par("lnqw", [1, DH], dt.float32r)
    lnqb = par("lnqb", [DH, 1])
    lnkw = par("lnkw", [1, DH], dt.float32r)
    lnkb = par("lnkb", [DH, 1])
    lnow = par("lnow", [1, DH], dt.float32r)
    lnob = par("lnob", [DH, 1])
    odiags = par("odiags", [DH, 9 * DH], dt.bfloat16)
    opw = par("opw", [DH, DIM], dt.bfloat16)
    ln2w = par("ln2w", [1, DIM])
    ln2b = par("ln2b", [1, DIM])
    o64hd = par("o64h", [DH, 1], dt.float32r)
    zpad = par("zpad", [DIM, G], dt.float32r)
    onesrd = par("onesr", [1, DH], dt.float32r)
    out_ext = nc.declare_dram_parameter("out", [N // 4, DIM], dt.float32, isOutput=True)

    rs_in = nc.dram_tensor("rs_in", [N, DIM], dt.float32)
    rs_out = nc.dram_tensor("rs_out", [N // 4, DIM], dt.float32)

    with (
        nc.allow_low_precision(reason="float32r/bf16 compute by design"),
        tile.TileContext(nc) as tc,
        tc.tile_pool(name="main", bufs=1) as main,
        tc.tile_pool(name="tmp2", bufs=2) as tmp2,
    ):
        # ---- persistent SBUF tiles ----
        QL = main.tile([128, N], dt.float32r)  # LN'd q, duplicated on both halves
        KL = main.tile([128, N], dt.float32r)
        VT = main.tile([DH, N], dt.bfloat16)  # v^T for the skip connection
        V = main.tile([128, NCH, DH + 1], dt.bfloat16)
        SC = main.tile([1, 2 * N], dt.float32)  # mu | E2 (E2 becomes var/rs)
        SCB = main.tile([1, 2 * N], dt.float32r)  # rs | mu*rs (matmul-ready)
        Og = main.tile([DH, G, G], dt.bfloat16)  # padded out-LN grid
        odg = main.tile([DH, 9, DH], dt.bfloat16)
        nc.scalar.dma_start(
            out=odg, in_=odiags[:, :].rearrange("p (t c) -> p t c", t=9)
        )
        opw_t = main.tile([DH, DIM], dt.bfloat16)
        nc.scalar.dma_start(out=opw_t, in_=opw[:, :])
        DWO = main.tile([DH, N], dt.bfloat16)
        o64h = main.tile([DH, 1], dt.float32r)
        nc.sync.dma_start(out=o64h, in_=o64hd[:, :])
        lnqb_t = main.tile([DH, 1], dt.float32)
        lnkb_t = main.tile([DH, 1], dt.float32)
        lnob_t = main.tile([DH, 1], dt.float32)
        nc.sync.dma_start(out=lnqb_t, in_=lnqb[:, :])
        nc.sync.dma_start(out=lnkb_t, in_=lnkb[:, :])
        nc.sync.dma_start(out=lnob_t, in_=lnob[:, :])
        lnqw_t = main.tile([1, DH], dt.float32r)
        lnkw_t = main.tile([1, DH], dt.float32r)
        lnow_t = main.tile([1, DH], dt.float32r)
        nc.sync.dma_start(out=lnqw_t, in_=lnqw[:, :])
        nc.sync.dma_start(out=lnkw_t, in_=lnkw[:, :])
        nc.sync.dma_start(out=lnow_t, in_=lnow[:, :])
        onesr = main.tile([1, DH], dt.float32r)
        nc.sync.dma_start(out=onesr, in_=onesrd[:, :])
        epsP = main.tile([128, 1], dt.float32)
        nc.vector.memset(epsP, EPS)
        nc.vector.memset(V, 1.0)
        nc.vector.memset(Og, 0.0)

        def stats_mms(psp, src_ap, c, shared=False):
            """mu and E[x^2] rows for a [64, IC] chunk into SC columns c."""
            sq = tmp2.tile([DH, IC], dt.float32r, tag="sq")
            nc.vector.tensor_mul(out=sq, in0=src_ap, in1=src_ap)
            smu = psp.tile([1, IC], dt.float32, tag="st" if shared else "smu", bufs=1)
            nc.tensor.matmul(smu, o64h, src_ap, start=True, stop=True)
            nc.vector.tensor_copy(out=SC[:, c * IC : (c + 1) * IC], in_=smu)
            se2 = psp.tile([1, IC], dt.float32, tag="st" if shared else "se2", bufs=1)
            nc.tensor.matmul(se2, o64h, sq, start=True, stop=True)
            nc.vector.tensor_copy(out=SC[:, N + c * IC : N + (c + 1) * IC], in_=se2)

        def ln_chain(lo, hi):
            """SC mu/E2 -> SCB rs / mu*rs over columns [lo, hi)."""
            mu = SC[:, lo:hi]
            e2 = SC[:, N + lo : N + hi]
            mrs = SCB[:, N + lo : N + hi]
            rs = SCB[:, lo:hi]
            nc.vector.scalar_tensor_tensor(
                out=mrs, in0=mu, scalar=-1.0, in1=mu, op0=OP.mult, op1=OP.mult
            )
            nc.vector.tensor_add(out=e2, in0=e2, in1=mrs)
            # 1/sqrt(v+eps) = exp(-0.5*ln(v+eps)): keeps the Exp table set
            # resident (a Sqrt would force a table reload every chunk)
            nc.scalar.activation(out=e2, in_=e2, func=AF.Ln, bias=epsP[0:1, :])
            nc.scalar.activation(out=rs, in_=e2, func=AF.Exp, scale=-0.5)
            nc.vector.tensor_mul(out=mrs, in0=mu, in1=rs)

        def ln_bcast_apply(psp, w_row, b_t, src_ap, dst, c, dst_is_grid=False, bc_bufs=2):
            """dst chunk c = (src*rs - mu*rs)*w + b via two K=1 bcasts."""
            bcA = psp.tile([DH, IC], dt.float32, tag="bc", bufs=bc_bufs)
            nc.tensor.matmul(
                bcA, w_row, SCB[:, c * IC : (c + 1) * IC], start=True, stop=True
            )
            bcB = psp.tile([DH, IC], dt.float32, tag="bc", bufs=bc_bufs)
            nc.tensor.matmul(
                bcB, w_row, SCB[:, N + c * IC : N + (c + 1) * IC], start=True, stop=True
            )
            T = tmp2.tile([DH, IC], dt.float32, tag="T")
            nc.vector.tensor_mul(out=T, in0=src_ap, in1=bcA)
            if dst_is_grid:
                r0 = c * 8
                nc.vector.scalar_tensor_tensor(
                    out=dst[:, 1 + r0 : 9 + r0, 1 : 1 + Ws],
                    in0=T.rearrange("p (a b) -> p a b", b=Ws),
                    scalar=b_t,
                    in1=bcB.rearrange("p (a b) -> p a b", b=Ws),
                    op0=OP.add,
                    op1=OP.subtract,
                )
            else:
                nc.vector.scalar_tensor_tensor(
                    out=dst[0:DH, c * IC : (c + 1) * IC],
                    in0=T,
                    scalar=b_t,
                    in1=bcB,
                    op0=OP.add,
                    op1=OP.subtract,
                )

        # ============ Stage A1: pos + depthwise ============
        with tc.tile_pool(name="stageA", bufs=1) as pA:
            psA1cm = tc.tile_pool(name="psA1", bufs=2, space="PSUM")
            psA1 = psA1cm.__enter__()
            pADWcm = tc.tile_pool(name="pADW", bufs=1)
            pADW = pADWcm.__enter__()
            Xg = pADW.tile([DIM, G, G], dt.float32r)
            pA0 = tc.tile_pool(name="pA0", bufs=1)
            pA0p = pA0.__enter__()
            pwt = pA0p.tile([3, DIM], dt.bfloat16)
            nc.scalar.dma_start(out=pwt, in_=pewT[:, :])
            nc.sync.dma_start(out=Xg[:, 0:1, :], in_=zpad[:, :].unsqueeze(1))
            nc.sync.dma_start(out=Xg[:, G - 1 : G, :], in_=zpad[:, :].unsqueeze(1))
            nc.scalar.dma_start(
                out=Xg[:, 1 : G - 1, 0:1], in_=zpad[:, 0 : G - 2].unsqueeze(2)
            )
            nc.scalar.dma_start(
                out=Xg[:, 1 : G - 1, G - 1 : G], in_=zpad[:, 0 : G - 2].unsqueeze(2)
            )
            for q in range(8):
                eng = nc.sync if q % 2 == 0 else nc.scalar
                eng.dma_start(
                    out=Xg[:, 1 + 8 * q : 1 + 8 * (q + 1), 1 : 1 + Ws],
                    in_=x[:, 512 * q : 512 * (q + 1)]
                    .bitcast(dt.float32r)
                    .rearrange("p (a b) -> p a b", b=Ws),
                )
            pwq_t = pA.tile([DIM, DH], dt.float32r)
            pwk_t = pA.tile([DIM, DH], dt.float32r)
            pwv_t = pA.tile([DIM, DH], dt.float32r)
            nc.sync.dma_start(out=pwq_t, in_=pwq[:, :])
            nc.sync.dma_start(out=pwk_t, in_=pwk[:, :])
            nc.sync.dma_start(out=pwv_t, in_=pwv[:, :])

            # pos encoding into the guard interior
            for c in range(NIC):
                mshc = tmp2.tile([3, IC], dt.bfloat16, tag="mshc", bufs=1)
                nc.scalar.dma_start(out=mshc, in_=meshb[:, c * IC : (c + 1) * IC])
                pos_ps = psA1.tile([DIM, IC], dt.float32, tag="pos")
                nc.tensor.matmul(pos_ps, pwt, mshc, start=True, stop=True)
                r0 = c * 8
                view = Xg[:, 1 + r0 : 9 + r0, 1 : 1 + Ws]
                nc.vector.tensor_add(
                    out=view, in0=view, in1=pos_ps.rearrange("p (a b) -> p a b", b=Ws)
                )

            pA0.__exit__(None, None, None)
            # depthwise 3x3 via 9 accumulated diag matmuls
            qdg = pADW.tile([DIM, 9, DIM], dt.float32r)
            nc.scalar.dma_start(
                out=qdg, in_=qdiags[:, :].rearrange("p (t c) -> p t c", t=9)
            )
            Yr = pA.tile([DIM, N], dt.float32r)
            for c in range(NIC):
                dwp = psA1.tile([DIM, IC], dt.float32, tag="dw")
                r0 = c * 8
                t = 0
                for di in range(3):
                    for dj in range(3):
                        nc.tensor.matmul(
                            dwp,
                            qdg[:, t, :],
                            Xg[:, r0 + di : r0 + di + 8, dj : dj + Ws],
                            start=(t == 0),
                            stop=(t == 8),
                        )
                        t += 1
                nc.vector.tensor_copy(out=Yr[:, c * IC : (c + 1) * IC], in_=dwp)

            psA1cm.__exit__(None, None, None)
            pADWcm.__exit__(None, None, None)
            # ============ Stage A2: pointwise + q/k LN + v ============
            with tc.tile_pool(name="psA2", bufs=1, space="PSUM") as psA2:
                QRW = pA.tile([DH, N], dt.float32r)
                KRAW = pA.tile([DH, N], dt.float32r)
                for c in range(NIC):
                    qp = psA2.tile([DH, IC], dt.float32, tag="qp", bufs=2)
                    nc.tensor.matmul(
                        qp, pwq_t, Yr[:, c * IC : (c + 1) * IC], start=True, stop=True
                    )
                    nc.vector.tensor_copy(out=QRW[:, c * IC : (c + 1) * IC], in_=qp)
                for c in range(NIC):
                    stats_mms(psA2, QRW[:, c * IC : (c + 1) * IC], c)
                # q chain on DVE/ACT runs while the PE does k's pointwise+stats
                ln_chain(0, N)
                for c in range(NIC):
                    qp = psA2.tile([DH, IC], dt.float32, tag="qp", bufs=2)
                    nc.tensor.matmul(
                        qp, pwk_t, Yr[:, c * IC : (c + 1) * IC], start=True, stop=True
                    )
                    nc.vector.tensor_copy(out=KRAW[:, c * IC : (c + 1) * IC], in_=qp)
                for c in range(NIC):
                    stats_mms(psA2, KRAW[:, c * IC : (c + 1) * IC], c)

                # q broadcast + apply -> QL lower half, then mirror upward
                for c in range(NIC):
                    ln_bcast_apply(psA2, lnqw_t, lnqb_t, QRW[:, c * IC : (c + 1) * IC], QL, c)
                nc.vector.tensor_copy(out=QL[DH:128, :], in_=QL[0:DH, :])

                # k chain (waits for q's SCB readers) overlaps v / v^T builds
                ln_chain(0, N)
                for ch in range(NCH):
                    vp = psA2.tile([128, DH], dt.float32, tag="vp", bufs=2)
                    nc.tensor.matmul(
                        vp,
                        Yr[:, ch * 128 : (ch + 1) * 128],
                        pwv_t,
                        start=True,
                        stop=True,
                    )
                    nc.vector.tensor_copy(out=V[:, ch, 0:DH], in_=vp)
                for c in range(NIC):
                    qp = psA2.tile([DH, IC], dt.float32, tag="qp", bufs=2)
                    nc.tensor.matmul(
                        qp, pwv_t, Yr[:, c * IC : (c + 1) * IC], start=True, stop=True
                    )
                    nc.vector.tensor_copy(out=VT[:, c * IC : (c + 1) * IC], in_=qp)

                for c in range(NIC):
                    ln_bcast_apply(psA2, lnkw_t, lnkb_t, KRAW[:, c * IC : (c + 1) * IC], KL, c)
                nc.vector.tensor_copy(out=KL[DH:128, :], in_=KL[0:DH, :])

        # ============ Stage B: attention with inline out-LN ============
        with tc.tile_pool(name="psB", bufs=1, space="PSUM") as psB, tc.tile_pool(
            name="sbB", bufs=3
        ) as sbB:
            NG = NJB // 2
            pending_tail = []

            def attention_block(c):
                avp = psB.tile([DH + 1, IC], dt.float32, tag="avp", bufs=1)
                stgs = {}
                Es = {}

                def issue_st(g):
                    stg = psB.tile([128, 2 * IC], dt.float32, tag="stg", bufs=2)
                    j0 = 2 * g * JB
                    nc.tensor.matmul(
                        stg[:, 0:IC],
                        KL[0:DH, j0 : j0 + JB],
                        QL[0:DH, c * IC : (c + 1) * IC],
                        start=True,
                        stop=True,
                    )
                    nc.tensor.matmul(
                        stg[:, IC : 2 * IC],
                        KL[DH:128, j0 + JB : j0 + 2 * JB],
                        QL[DH:128, c * IC : (c + 1) * IC],
                        start=True,
                        stop=True,
                    )
                    stgs[g] = stg

                def issue_exp(g):
                    E = sbB.tile([128, 2 * IC], dt.bfloat16, tag="E")
                    nc.scalar.activation(
                        out=E, in_=stgs.pop(g), func=AF.Exp, scale=float(DH**-0.5)
                    )
                    Es[g] = E

                def issue_av(g):
                    E = Es.pop(g)
                    for t in range(2):
                        jb = 2 * g + t
                        nc.tensor.matmul(
                            avp,
                            V[:, jb, :],
                            E[:, t * IC : (t + 1) * IC],
                            start=(jb == 0),
                            stop=(jb == NJB - 1),
                            skip_group_check=True,
                        )

                issue_st(0)
                issue_exp(0)
                for g in range(1, NG):
                    issue_st(g)
                    issue_exp(g)
                    issue_av(g - 1)
                issue_av(NG - 1)

                # free avp quickly: park the numerator and denominator
                REC = sbB.tile([1, IC], dt.float32, tag="REC", bufs=2)
                nc.vector.tensor_copy(out=REC, in_=avp[DH : DH + 1, :])
                Tn = sbB.tile([DH, IC], dt.float32, tag="Tn", bufs=2)
                nc.vector.tensor_copy(out=Tn, in_=avp[0:DH, :])
                return REC, Tn

            def tail_block(c, REC, Tn):
                # DVE/ACT chain + the few tail matmuls for chunk c; issued
                # after the NEXT chunk's attention matmuls so the PE stream
                # never waits on the chain.
                nc.vector.reciprocal_approx_fast(out=REC, in_=REC)
                RECB = sbB.tile([1, IC], dt.float32r, tag="RECB", bufs=2)
                nc.vector.tensor_copy(out=RECB, in_=REC)
                bcR = psB.tile([DH, IC], dt.float32, tag="bc", bufs=1)
                nc.tensor.matmul(bcR, onesr, RECB, start=True, stop=True)
                OSc = sbB.tile([DH, IC], dt.float32r, tag="OS", bufs=2)
                nc.vector.tensor_mul(out=OSc, in0=Tn, in1=bcR)
                nc.vector.tensor_add(
                    out=OSc, in0=OSc, in1=VT[:, c * IC : (c + 1) * IC]
                )
                stats_mms(psB, OSc[:, :], c, shared=True)
                ln_chain(c * IC, (c + 1) * IC)
                ln_bcast_apply(
                    psB, lnow_t, lnob_t, OSc[:, :], Og, c, dst_is_grid=True, bc_bufs=1
                )

            def dw_chunk(c):
                dwp = psB.tile([DH, IC], dt.float32, tag="dw", bufs=1)
                r0 = c * 8
                t = 0
                for di in range(3):
                    for dj in range(3):
                        nc.tensor.matmul(
                            dwp,
                            odg[:, t, :],
                            Og[:, r0 + di : r0 + di + 8, dj : dj + Ws],
                            start=(t == 0),
                            stop=(t == 8),
                        )
                        t += 1
                nc.vector.tensor_copy(out=DWO[:, c * IC : (c + 1) * IC], in_=dwp)

            for c in range(NIC):
                rec_tn = attention_block(c)
                if pending_tail:
                    tail_block(*pending_tail.pop())
                if c >= 2:
                    dw_chunk(c - 2)
                pending_tail.append((c, *rec_tn))
            tail_block(*pending_tail.pop())
            dw_chunk(NIC - 2)
            dw_chunk(NIC - 1)

        # ============ Stage C: out depthwise + pointwise partial ============
        with tc.tile_pool(name="psC", bufs=2, space="PSUM") as psC:
            for ch in range(NCH):
                pp = psC.tile([128, DIM], dt.float32, tag="pp")
                nc.tensor.matmul(
                    pp, DWO[:, ch * 128 : (ch + 1) * 128], opw_t, start=True, stop=True
                )
                PP = tmp2.tile([128, DIM], dt.float32, tag="PP")
                nc.vector.tensor_copy(out=PP, in_=pp)
                nc.sync.dma_start(out=rs_in[ch * 128 : (ch + 1) * 128, :], in_=PP)

        # ============ Stage D: ReduceScatter + LayerNorm2d ============
        nc.gpsimd.collective_compute(
            "ReduceScatter",
            OP.add,
            replica_groups=[[0, 1, 2, 3], [4, 5, 6, 7]],
            ins=[rs_in[:, :]],
            outs=[rs_out[:, :]],
        )
        with tc.tile_pool(name="stageD", bufs=2) as pD:
            w_b = pD.tile([128, DIM], dt.float32, bufs=1)
            b_b = pD.tile([128, DIM], dt.float32, bufs=1)
            nc.sync.dma_start(out=w_b, in_=ln2w[:, :].to_broadcast([128, DIM]))
            nc.sync.dma_start(out=b_b, in_=ln2b[:, :].to_broadcast([128, DIM]))
            for tkn in range(8):
                R = pD.tile([128, DIM], dt.float32, tag="R")
                nc.sync.dma_start(out=R, in_=rs_out[tkn * 128 : (tkn + 1) * 128, :])
                st = pD.tile([128, 6], dt.float32, tag="st")
                nc.vector.bn_stats(out=st, in_=R)
                mv = pD.tile([128, 2], dt.float32, tag="mv")
                nc.vector.bn_aggr(out=mv, in_=st)
                sd = pD.tile([128, 1], dt.float32, tag="sd")
                nc.scalar.activation(out=sd, in_=mv[:, 1:2], func=AF.Ln, bias=epsP)
                nc.scalar.activation(out=sd, in_=sd, func=AF.Exp, scale=-0.5)
                nc.vector.tensor_scalar(
                    out=R,
                    in0=R,
                    scalar1=mv[:, 0:1],
                    scalar2=sd,
                    op0=OP.subtract,
                    op1=OP.mult,
                )
                R2 = pD.tile([128, DIM], dt.float32, tag="R2")
                nc.vector.tensor_mul(out=R2, in0=R, in1=w_b)
                nc.vector.tensor_add(out=R2, in0=R2, in1=b_b)
                nc.sync.dma_start(out=out_ext[tkn * 128 : (tkn + 1) * 128, :], in_=R2)

    return nc


_cached = {}


def _get_nc():
    if "nc" not in _cached:
        nc = _build()
        nc.finalize()
        _cached["nc"] = nc
    return _cached["nc"]


def _make_in_maps(inputs):
    import ml_dtypes

    x = np.asarray(inputs["x"], np.float32)
    pe_w = np.asarray(inputs["pe_w"], np.float32)
    pe_b = np.asarray(inputs["pe_b"], np.float32)
    qkv_dw = np.asarray(inputs["qkv_dw"], np.float32)
    qkv_pw = np.asarray(inputs["qkv_pw"], np.float32)
    out_dw = np.asarray(inputs["out_dw"], np.float32)
    out_pw = np.asarray(inputs["out_pw"], np.float32)
    nq_w, nq_b = np.asarray(inputs["nq_w"], np.float32), np.asarray(
        inputs["nq_b"], np.float32
    )
    nk_w, nk_b = np.asarray(inputs["nk_w"], np.float32), np.asarray(
        inputs["nk_b"], np.float32
    )
    no_w, no_b = np.asarray(inputs["no_w"], np.float32), np.asarray(
        inputs["no_b"], np.float32
    )
    ln_w, ln_b = np.asarray(inputs["ln_w"], np.float32), np.asarray(
        inputs["ln_b"], np.float32
    )

    gx = np.linspace(0.0, 1.0, Hs, dtype=np.float32)
    gy = np.linspace(0.0, 1.0, Ws, dtype=np.float32)
    meshb = np.stack(
        [np.repeat(gx, Ws), np.tile(gy, Hs), np.ones(N, np.float32)]
    ).astype(ml_dtypes.bfloat16)
    pewT = np.stack([pe_w[:, 0], pe_w[:, 1], pe_b]).astype(ml_dtypes.bfloat16)

    idx = np.arange(DH)
    in_maps = []
    for c in range(8):
        b, h = c // 4, c % 4
        rows = h + HEADS * idx
        qdiags = np.zeros((DIM, 9, DIM), np.float32)
        taps = qkv_dw.reshape(DIM, 9)
        for t in range(9):
            qdiags[np.arange(DIM), t, np.arange(DIM)] = taps[:, t]
        odiags = np.zeros((DH, 9, DH), np.float32)
        otaps = out_dw[rows].reshape(DH, 9)
        for t in range(9):
            odiags[idx, t, idx] = otaps[:, t]
        m = {
            "x": np.ascontiguousarray(x[b].reshape(DIM, N)),
            "meshb": meshb,
            "pewT": pewT,
            "qdiags": np.ascontiguousarray(qdiags.reshape(DIM, 9 * DIM)),
            "pwq": np.ascontiguousarray(qkv_pw[rows, :].T),
            "pwk": np.ascontiguousarray(qkv_pw[DIM * 2 + rows, :].T),
            "pwv": np.ascontiguousarray(qkv_pw[DIM * 4 + rows, :].T),
            "lnqw": np.ascontiguousarray(nq_w[h][None, :]),
            "lnqb": np.ascontiguousarray(nq_b[h][:, None]),
            "lnkw": np.ascontiguousarray(nk_w[h][None, :]),
            "lnkb": np.ascontiguousarray(nk_b[h][:, None]),
            "lnow": np.ascontiguousarray(no_w[h][None, :]),
            "lnob": np.ascontiguousarray(no_b[h][:, None]),
            "odiags": np.ascontiguousarray(odiags.reshape(DH, 9 * DH)).astype(
                ml_dtypes.bfloat16
            ),
            "opw": np.ascontiguousarray(out_pw[:, rows].T).astype(ml_dtypes.bfloat16),
            "ln2w": np.ascontiguousarray(ln_w[None, :]),
            "ln2b": np.ascontiguousarray(ln_b[None, :]),
            "o64h": np.full((DH, 1), 1.0 / DH, np.float32),
            "zpad": np.zeros((DIM, G), np.float32),
            "onesr": np.ones((1, DH), np.float32),
        }
        in_maps.append(m)
    return in_maps


def run_on_device(inputs, **kw):
    nc = _get_nc()
    in_maps = _make_in_maps(inputs)
    res = run_bass_kernel_spmd(nc, in_maps, core_ids=list(range(8)), **kw)
    out = np.zeros((B, DIM, N), np.float32)
    for c in range(8):
        b, h = c // 4, c % 4
        out[b][:, h * (N // 4) : (h + 1) * (N // 4)] = res.results[c]["out"].T
    return out.reshape(B, DIM, Hs, Ws), res


def kernel(**inputs):
    out, _ = run_on_device(inputs)
    return out
